# revision 25
# baseline (speedup 1.0000x reference)
"""MetaQuickSR Trainium2 kernel (8-core SPMD, row-sharded), v2.

Sharding: H=256 output-feature rows split 32/core (+4-row conv halo).
Each core: 4-layer CNN (block-diagonal image batching on PE) -> PE-based
im2col row transposes -> bf16 Pos2Weight MLP -> per-pixel locally-
connected einsum split across DVE+Pool -> transpose/interleave writeback
with contiguous output DMAs.  No cross-core communication.
"""

import numpy as np
import ml_dtypes

import concourse.bass as bass
import concourse.mybir as mybir
from concourse.tile import TileContext
from concourse.bass_utils import run_bass_kernel_spmd
from concourse.masks import make_identity

BF16 = ml_dtypes.bfloat16

NCORES = 8
N, CI, Himg, Wimg, S = 4, 16, 256, 256, 2
ROWS = Himg // NCORES          # 32 output-feature rows per core
HALO = 4
NR = ROWS + 2 * HALO           # 40 buffered rows
WP = Wimg + 2                  # 258 zero-padded width
NPIX = ROWS * Wimg             # 8192 einsum pixels per core
NT = NPIX // 128               # 64 pixel tiles per q plane
RGB_MEAN = (0.4488, 0.4371, 0.404)
RGB_RANGE = 255.0

XW = NR * WP + 256             # dense x (12 parts) + w1 rows 0-2
WW = 4 * 9 * 128 + 2 * 432 + 432   # cwB + w2p + b2p(row0)

# einsum reduction split (after a Pool ci-fold halves the volume):
# DVE seg-reduces c < DVE_C in one op plus n < EXTRA_N of c=DVE_C in a
# second; ACT accum-copies the remaining (c, n) pairs.
DVE_C = 2
EXTRA_N = 2

_NC = None


def _legalize_waits(nc, lim=1):
    """This walrus build accepts only one sync-wait per instruction; move
    surplus waits onto same-engine NoOps inserted just before."""
    cnt = 0
    for f in nc.m.functions:
        for bb in f.blocks:
            new = []
            for inst in bb.instructions:
                si = inst.sync_info
                if si is not None and si.on_wait is not None \
                        and len(si.on_wait) > lim:
                    waits = list(si.on_wait)
                    excess, keep = waits[:-lim], waits[-lim:]
                    for w in excess:
                        cnt += 1
                        nop = mybir.InstNoOp(
                            name=f"I-lw{cnt}", opcode="NoOp",
                            engine=inst.engine, debug=inst.debug,
                            ins=[], outs=[],
                            sync_info=mybir.SyncInfo(on_wait=[w],
                                                     on_update=[]))
                        new.append(nop)
                        nc.inst_map[nop.name] = nop
                    inst.sync_info = mybir.SyncInfo(
                        on_wait=keep, on_update=list(si.on_update or []))
                new.append(inst)
            bb.instructions = new
    return cnt


def _build_program():
    nc = bass.Bass(trn_type="TRN2")
    f32 = mybir.dt.float32
    bf = mybir.dt.bfloat16

    xin = nc.dram_tensor("xin", [12, XW], bf, kind="ExternalInput")
    win = nc.dram_tensor("win", [128, WW], bf, kind="ExternalInput")
    fin32 = nc.dram_tensor("fin32", [128, 12], f32, kind="ExternalInput")
    post = nc.dram_tensor("post", [4, 3, NPIX], bf, kind="ExternalInput")
    outd = nc.dram_tensor("out", [4, 3, 2 * ROWS, 2 * Wimg], f32,
                          kind="ExternalOutput")

    mul = mybir.AluOpType.mult

    with TileContext(nc) as tc:
        with (
            tc.tile_pool(name="singles", bufs=1) as singles,
            tc.tile_pool(name="pos_p", bufs=2) as pos_p,
            tc.tile_pool(name="ht_p", bufs=2) as ht_p,
            tc.tile_pool(name="lws_p", bufs=3) as lws_p,
            tc.tile_pool(name="scr_p", bufs=3) as scr_p,
            tc.tile_pool(name="scrf_p", bufs=3) as scrf_p,
            tc.tile_pool(name="aj_p", bufs=2) as aj_p,
        ):
            # ---- resident tiles --------------------------------------
            xw_sb = singles.tile([12, XW], bf)
            win_sb = singles.tile([128, WW], bf)
            f32_sb = singles.tile([128, 12], f32)
            fA = singles.tile([128, NR, WP], bf)
            fB = singles.tile([128, NR, WP], bf)
            f4c = singles.tile([64, NR, WP], bf)
            # fT2h[hf][pix, (row 34, kw 3, (n,ci) 64)]
            fT2h = [singles.tile([128, 34 * 3 * 64], bf, name=f"fT2h{h}")
                    for h in range(2)]
            outq = [singles.tile([128, 768], f32, name=f"outq{q}")
                    for q in range(4)]
            staged = [singles.tile([128, 6, 256], f32, name=f"stg{s}")
                      for s in range(2)]
            ones_sb = singles.tile([1, 128], bf)
            idbf = singles.tile([64, 64], bf)
            idf32 = singles.tile([128, 128], f32)
            dummy = singles.tile([1, 16], bf)

            nc.sync.dma_start(xw_sb[:, :], xin[:, :])
            nc.scalar.dma_start(win_sb[:, :], win[:, :])
            nc.scalar.dma_start(f32_sb[:, :], fin32[:, :])
            nc.gpsimd.memset(fA[:, :, :], 0.0)
            nc.gpsimd.memset(fB[:, :, :], 0.0)
            nc.gpsimd.memset(ones_sb[:, :], 1.0)
            nc.gpsimd.memset(staged[0][:, :, :], 0.0)
            nc.gpsimd.memset(staged[1][:, :, :], 0.0)
            make_identity(nc, idbf)
            make_identity(nc, idf32)

            xv = xw_sb[:, 0:NR * WP].rearrange("p (r w) -> p r w", w=WP)
            w1v = xw_sb[0:3, NR * WP:NR * WP + 256]
            cw = win_sb[:, 0:4608].rearrange("p (l t o) -> p l t o",
                                             t=9, o=128)
            w2pv = win_sb[:, 4608:4608 + 864].rearrange(
                "p (j c) -> p j c", c=432)
            b2pv = win_sb[0:1, 5472:5904]
            cb = f32_sb[:, 0:4]
            b1c = f32_sb[:, 4:6]
            shiftv = f32_sb[:, 6:12]

            # warm ACT's vector clock (1 wait per op) so conv relu-copies
            # only ever wait on PE.
            nc.scalar.copy(dummy[0:1, 0:1], xw_sb[0:1, 0:1])
            nc.scalar.copy(dummy[0:1, 1:2], win_sb[0:1, 0:1])
            nc.scalar.copy(dummy[0:1, 2:3], fA[0:1, 0:1, 0:1])
            nc.scalar.copy(dummy[0:1, 3:4], fB[0:1, 0:1, 0:1])

            # ---- conv chain + interleaved im2col ---------------------
            # l: 0:x->fA  1:fA->fB  2:fB->fA  3:fA->fB
            fins = [xv, fA, fB, fA]
            fouts = [fA, fB, fA, fB]

            def compact_rows(r0, r1):
                for n in range(4):
                    nc.sync.dma_start(
                        out=f4c[16 * n:16 * n + 16, r0:r1, :],
                        in_=fB[32 * n:32 * n + 16, r0:r1, :])

            def transpose_rows(rr):
                # r in fT2 coords (f4 row = r+3)
                for r in rr:
                    for hf in range(2):
                        tp = tps.tile([128, 3, 64], bf, tag="tps")
                        for kw in range(3):
                            nc.tensor.transpose(
                                tp[:, kw, :],
                                f4c[:, r + 3, 128 * hf + kw:
                                    128 * hf + kw + 128],
                                idbf[:, :])
                        nc.vector.tensor_copy(
                            fT2h[hf][:, 3 * r * 64:3 * (r + 1) * 64],
                            tp[:, :, :])

            with tc.tile_pool(name="cps", bufs=2, space="PSUM") as cps, \
                 tc.tile_pool(name="tps", bufs=3, space="PSUM") as tps:
                for l in range(4):
                    fin, fout = fins[l], fouts[l]
                    for ch in range(19):
                        r0 = 1 + 2 * ch
                        ps = cps.tile([128, 2, 256], f32, tag="convps")
                        for tap in range(9):
                            kh, kw = tap // 3, tap % 3
                            if l == 0:
                                lhsT = cw[0:12, 0, tap, :]
                                rhs = fin[0:12, r0 + kh - 1:r0 + kh + 1,
                                          kw:kw + 256]
                            else:
                                lhsT = cw[:, l, tap, :]
                                rhs = fin[:, r0 + kh - 1:r0 + kh + 1,
                                          kw:kw + 256]
                            nc.tensor.matmul(
                                ps[:, :, :], lhsT, rhs,
                                start=(tap == 0), stop=(tap == 8))
                        nc.scalar.activation(
                            fout[:, r0:r0 + 2, 1:257], ps[:, :, :],
                            mybir.ActivationFunctionType.Relu,
                            bias=cb[:, l:l + 1], scale=1.0)
                        # layer 3: compact + transpose finished row groups
                        if l == 3:
                            if ch == 7:
                                compact_rows(3, 13)      # f4 rows 3-12
                            elif ch == 9:
                                transpose_rows(range(0, 10))
                            elif ch == 12:
                                compact_rows(13, 23)
                            elif ch == 14:
                                transpose_rows(range(10, 20))
                            elif ch == 17:
                                compact_rows(23, 33)
                    if l == 3:
                        compact_rows(33, 37)
                        transpose_rows(range(20, 34))

            # ---- per-q: h MLP, local weights, einsum, writeback ------
            with tc.tile_pool(name="hps", bufs=2, space="PSUM") as hps, \
                 tc.tile_pool(name="lps", bufs=2, space="PSUM") as lps, \
                 tc.tile_pool(name="wps", bufs=2, space="PSUM") as wps:
                fT2v = [t.rearrange("p (t x) -> p t x", x=64)
                        for t in fT2h]
                for q in range(4):
                    si, sj = q // 2, q % 2
                    for pc in range(8):
                        pos_t = pos_p.tile([3, 1024], bf, tag="pos")
                        nc.scalar.dma_start(
                            pos_t[:, :],
                            post[q, :, pc * 1024:(pc + 1) * 1024])
                        hT = ht_p.tile([128, 2, 1024], bf, tag="ht")
                        for jh in range(2):
                            for hf2 in range(2):
                                hp = hps.tile([128, 512], f32, tag="hps")
                                nc.tensor.matmul(
                                    hp[:, :],
                                    w1v[:, jh * 128:(jh + 1) * 128],
                                    pos_t[:, hf2 * 512:(hf2 + 1) * 512],
                                    start=True, stop=True)
                                nc.scalar.activation(
                                    hT[:, jh, hf2 * 512:(hf2 + 1) * 512],
                                    hp[:, :],
                                    mybir.ActivationFunctionType.Relu,
                                    bias=b1c[:, jh:jh + 1], scale=1.0)
                        for tl in range(8):
                            t = pc * 8 + tl
                            r0, hf = t // 2, t % 2
                            lwp = lps.tile([128, 3, 9, 16], f32,
                                           tag="lwp")
                            for jh in range(2):
                                nc.tensor.matmul(
                                    lwp[:, :, :, :],
                                    hT[:, jh, tl * 128:(tl + 1) * 128],
                                    w2pv[:, jh, :],
                                    start=(jh == 0), stop=False)
                            nc.tensor.matmul(
                                lwp[:, :, :, :], ones_sb[:, :], b2pv,
                                start=False, stop=True)
                            lws = lws_p.tile([128, 3, 9, 16], bf,
                                             tag="lws")
                            nc.scalar.activation(
                                lws[:, :, :, :], lwp[:, :, :, :],
                                mybir.ActivationFunctionType.Copy)
                            # DVE: broadcast products (per c; n bcast on
                            # in1) -- TT APs allow at most 3 free dims
                            scr2 = scr_p.tile([128, 3, 4, 9, 16], bf,
                                              tag="scr2")
                            in0 = fT2v[hf][:, 3 * r0:3 * r0 + 9, :] \
                                .rearrange("p t (n i) -> p t n i", n=4) \
                                .transpose([0, 2, 1, 3])
                            for c in range(3):
                                in1 = lws[:, c, :, :].unsqueeze(1) \
                                    .broadcast_to([128, 4, 9, 16])
                                nc.vector.tensor_tensor(
                                    out=scr2[:, c, :, :, :],
                                    in0=in0, in1=in1, op=mul)
                            # Pool: fold ci halves (144 -> 72 per pair);
                            # merge (c,n) -> 3 free dims for the Pool ISA
                            scrf = scrf_p.tile([128, 12, 9, 8], bf,
                                               tag="scrf")
                            s2m = scr2.rearrange(
                                "p c n t i -> p (c n) t i")
                            nc.gpsimd.tensor_tensor(
                                out=scrf[:, :, :, :],
                                in0=s2m[:, :, :, 0:8],
                                in1=s2m[:, :, :, 8:16],
                                op=mybir.AluOpType.add)
                            scrfv = scrf.rearrange(
                                "p (c n) t i -> p c n t i", c=3)
                            # DVE: segmented reduces
                            oqv = outq[q].rearrange(
                                "p (n c t) -> p c n t", c=3, t=64)
                            nc.vector.tensor_reduce(
                                out=oqv[:, 0:DVE_C, :, t],
                                in_=scrfv[:, 0:DVE_C, :, :, :],
                                axis=mybir.AxisListType.XY,
                                op=mybir.AluOpType.add)
                            if EXTRA_N:
                                nc.vector.tensor_reduce(
                                    out=oqv[:, DVE_C:DVE_C + 1,
                                            0:EXTRA_N, t],
                                    in_=scrfv[:, DVE_C, 0:EXTRA_N, :, :]
                                    .unsqueeze(1),
                                    axis=mybir.AxisListType.XY,
                                    op=mybir.AluOpType.add)
                            # ACT: accum-copies for the rest
                            for n in range(EXTRA_N, 4):
                                c = DVE_C
                                aj = aj_p.tile([128, 9, 8], bf,
                                               tag="aj")
                                nc.scalar.activation(
                                    aj[:, :, :],
                                    scrfv[:, c, n, :, :],
                                    mybir.ActivationFunctionType.Copy,
                                    accum_out=outq[q][
                                        :, (n * 3 + c) * 64 + t:
                                        (n * 3 + c) * 64 + t + 1])
                    # writeback: transpose + sj-interleave (+mean shift)
                    for j in range(6):
                        tq = wps.tile([128, 128], f32, tag="wps")
                        nc.tensor.transpose(
                            tq[:, :], outq[q][:, 128 * j:128 * (j + 1)],
                            idf32[:, :])
                        nc.scalar.activation(
                            staged[si].rearrange(
                                "p j (w s) -> p j w s", s=2)[:, j, :, sj],
                            tq[:, :],
                            mybir.ActivationFunctionType.Identity,
                            bias=shiftv[:, j:j + 1], scale=1.0)
                    if sj == 1:
                        # src partitions walk (a=nci_lo, r, h)-major then w;
                        # dst dims [a, r, h, w] match that element order.
                        dstv = outd.rearrange(
                            "n c (r s) (h w) -> (n c) s r h w",
                            s=2, h=2)
                        for j in range(6):
                            nc.sync.dma_start(
                                out=dstv[2 * j:2 * j + 2, si, :, :, :],
                                in_=staged[si][:, j, :])
    _legalize_waits(nc)
    return nc


def _get_nc():
    global _NC
    if _NC is None:
        _NC = _build_program()
    return _NC


def _prep_inputs(x, pos_mat, c0w, c0b, c1w, c1b, c2w, c2b, c3w, c3b,
                 w1, b1, w2, b2):
    """Host-side packing of per-core input dicts."""
    x = np.asarray(x, np.float32)
    pos = np.asarray(pos_mat, np.float32).reshape(-1, 3)

    # block-diagonal conv weights cwB[p, l, tap, 32n+co]
    cwB = np.zeros((128, 4, 9, 128), np.float32)
    cbp = np.zeros((128, 4), np.float32)
    for l, (wl, bl) in enumerate(((c0w, c0b), (c1w, c1b),
                                  (c2w, c2b), (c3w, c3b))):
        wl = np.asarray(wl, np.float32)          # (co, ci, 3, 3)
        K = wl.shape[1]
        t = wl.transpose(1, 2, 3, 0).reshape(K, 9, 16)   # (ci, tap, co)
        for n in range(4):
            if l == 0:
                cwB[3 * n:3 * n + K, l, :, 32 * n:32 * n + 16] = t
            else:
                cwB[32 * n:32 * n + K, l, :, 32 * n:32 * n + 16] = t
            cbp[32 * n:32 * n + 16, l] = np.asarray(bl, np.float32)

    w1 = np.asarray(w1, np.float32)              # (3, 256)
    b1p = np.asarray(b1, np.float32).reshape(2, 128).T.copy()  # [j, jh]

    # w2 columns: orig (s=ci*9+tap, c) -> permuted (c, tap, ci)
    w2 = np.asarray(w2, np.float32).reshape(256, 16, 9, 3)     # j,ci,tap,c
    w2pm = w2.transpose(0, 3, 2, 1).reshape(256, 432)          # j,(c,t,ci)
    w2pk = w2pm.reshape(2, 128, 432)                           # [jh,j,432]
    w2pk = np.ascontiguousarray(w2pk.transpose(1, 0, 2))       # [j,jh,432]
    b2 = np.asarray(b2, np.float32).reshape(16, 9, 3)
    b2pk = b2.transpose(2, 1, 0).reshape(432)

    # win pack: [cwB | w2p | b2p]
    winpk = np.zeros((128, WW), np.float32)
    winpk[:, 0:4608] = cwB.reshape(128, 4608)
    winpk[:, 4608:5472] = w2pk.reshape(128, 864)
    winpk[:, 5472:5904] = b2pk[None, :]

    # fin32: [cb | b1c | shift(j)]
    f32pk = np.zeros((128, 12), np.float32)
    f32pk[:, 0:4] = cbp
    f32pk[:, 4:6] = b1p
    for j in range(6):
        for p in range(128):
            nci = 2 * j + (1 if p >= 64 else 0)
            f32pk[p, 6 + j] = RGB_RANGE * RGB_MEAN[nci % 3]

    # pos rows ordered (h, si, w, sj); per-core chunk -> (q, 3, NPIX)
    posr = pos.reshape(Himg, 2, Wimg, 2, 3)

    in_maps = []
    for core in range(NCORES):
        h0 = core * ROWS
        xh = np.zeros((12, NR, WP), np.float32)
        lo, hi = h0 - HALO, h0 + ROWS + HALO
        slo, shi = max(lo, 0), min(hi, Himg)
        for n in range(4):
            xh[3 * n:3 * n + 3, slo - lo:shi - lo, 1:257] = \
                x[n, :, slo:shi, :]
        xpack = np.zeros((12, XW), np.float32)
        xpack[:, :NR * WP] = xh.reshape(12, -1)
        xpack[0:3, NR * WP:] = w1
        pc = posr[h0:h0 + ROWS].transpose(1, 3, 4, 0, 2)  # si,sj,3,h,w
        pc = pc.reshape(4, 3, NPIX)
        in_maps.append({
            "xin": xpack.astype(BF16),
            "win": winpk.astype(BF16),
            "fin32": f32pk,
            "post": np.ascontiguousarray(pc).astype(BF16),
        })
    return in_maps


LAST_RESULTS = None
TRACE = False


def kernel(**inputs):
    global LAST_RESULTS
    nc = _get_nc()
    in_maps = _prep_inputs(**inputs)
    res = run_bass_kernel_spmd(nc, in_maps, core_ids=list(range(NCORES)),
                               trace=TRACE)
    LAST_RESULTS = res
    out = np.concatenate([res.results[i]["out"] for i in range(NCORES)],
                         axis=2)
    return out.astype(np.float32)


# revision 26
# speedup vs baseline: 1.0015x; 1.0015x over previous
"""MetaQuickSR Trainium2 kernel (8-core SPMD, row-sharded), v2.

Sharding: H=256 output-feature rows split 32/core (+4-row conv halo).
Each core: 4-layer CNN (block-diagonal image batching on PE) -> PE-based
im2col row transposes -> bf16 Pos2Weight MLP -> per-pixel locally-
connected einsum split across DVE+Pool -> transpose/interleave writeback
with contiguous output DMAs.  No cross-core communication.
"""

import numpy as np
import ml_dtypes

import concourse.bass as bass
import concourse.mybir as mybir
from concourse.tile import TileContext
from concourse.bass_utils import run_bass_kernel_spmd
from concourse.masks import make_identity

BF16 = ml_dtypes.bfloat16

NCORES = 8
N, CI, Himg, Wimg, S = 4, 16, 256, 256, 2
ROWS = Himg // NCORES          # 32 output-feature rows per core
HALO = 4
NR = ROWS + 2 * HALO           # 40 buffered rows
WP = Wimg + 2                  # 258 zero-padded width
NPIX = ROWS * Wimg             # 8192 einsum pixels per core
NT = NPIX // 128               # 64 pixel tiles per q plane
RGB_MEAN = (0.4488, 0.4371, 0.404)
RGB_RANGE = 255.0

XW = NR * WP + 256             # dense x (12 parts) + w1 rows 0-2
WW = 4 * 9 * 128 + 2 * 432 + 432   # cwB + w2p + b2p(row0)

# einsum reduction split (after a Pool ci-fold halves the volume):
# DVE seg-reduces c < DVE_C in one op plus n < EXTRA_N of c=DVE_C in a
# second; ACT accum-copies the remaining (c, n) pairs.
DVE_C = 2
EXTRA_N = 2

_NC = None


def _legalize_waits(nc, lim=1):
    """This walrus build accepts only one sync-wait per instruction; move
    surplus waits onto same-engine NoOps inserted just before."""
    cnt = 0
    for f in nc.m.functions:
        for bb in f.blocks:
            new = []
            for inst in bb.instructions:
                si = inst.sync_info
                if si is not None and si.on_wait is not None \
                        and len(si.on_wait) > lim:
                    waits = list(si.on_wait)
                    excess, keep = waits[:-lim], waits[-lim:]
                    for w in excess:
                        cnt += 1
                        nop = mybir.InstNoOp(
                            name=f"I-lw{cnt}", opcode="NoOp",
                            engine=inst.engine, debug=inst.debug,
                            ins=[], outs=[],
                            sync_info=mybir.SyncInfo(on_wait=[w],
                                                     on_update=[]))
                        new.append(nop)
                        nc.inst_map[nop.name] = nop
                    inst.sync_info = mybir.SyncInfo(
                        on_wait=keep, on_update=list(si.on_update or []))
                new.append(inst)
            bb.instructions = new
    return cnt


def _build_program():
    nc = bass.Bass(trn_type="TRN2")
    f32 = mybir.dt.float32
    bf = mybir.dt.bfloat16

    xin = nc.dram_tensor("xin", [12, XW], bf, kind="ExternalInput")
    win = nc.dram_tensor("win", [128, WW], bf, kind="ExternalInput")
    fin32 = nc.dram_tensor("fin32", [128, 12], f32, kind="ExternalInput")
    post = nc.dram_tensor("post", [4, 3, NPIX], bf, kind="ExternalInput")
    outd = nc.dram_tensor("out", [4, 3, 2 * ROWS, 2 * Wimg], f32,
                          kind="ExternalOutput")

    mul = mybir.AluOpType.mult

    with TileContext(nc) as tc:
        with (
            tc.tile_pool(name="singles", bufs=1) as singles,
            tc.tile_pool(name="pos_p", bufs=2) as pos_p,
            tc.tile_pool(name="ht_p", bufs=2) as ht_p,
            tc.tile_pool(name="lws_p", bufs=4) as lws_p,
            tc.tile_pool(name="scr_p", bufs=4) as scr_p,
            tc.tile_pool(name="scrf_p", bufs=4) as scrf_p,
            tc.tile_pool(name="aj_p", bufs=4) as aj_p,
        ):
            # ---- resident tiles --------------------------------------
            xw_sb = singles.tile([12, XW], bf)
            win_sb = singles.tile([128, WW], bf)
            f32_sb = singles.tile([128, 12], f32)
            fA = singles.tile([128, NR, WP], bf)
            fB = singles.tile([128, NR, WP], bf)
            f4c = singles.tile([64, NR, WP], bf)
            # fT2h[hf][pix, (row 34, kw 3, (n,ci) 64)]
            fT2h = [singles.tile([128, 34 * 3 * 64], bf, name=f"fT2h{h}")
                    for h in range(2)]
            outq = [singles.tile([128, 768], f32, name=f"outq{q}")
                    for q in range(4)]
            staged = [singles.tile([128, 6, 256], f32, name=f"stg{s}")
                      for s in range(2)]
            ones_sb = singles.tile([1, 128], bf)
            idbf = singles.tile([64, 64], bf)
            idf32 = singles.tile([128, 128], f32)
            dummy = singles.tile([1, 16], bf)

            nc.sync.dma_start(xw_sb[:, :], xin[:, :])
            nc.scalar.dma_start(win_sb[:, :], win[:, :])
            nc.scalar.dma_start(f32_sb[:, :], fin32[:, :])
            nc.gpsimd.memset(fA[:, :, :], 0.0)
            nc.gpsimd.memset(fB[:, :, :], 0.0)
            nc.gpsimd.memset(ones_sb[:, :], 1.0)
            nc.gpsimd.memset(staged[0][:, :, :], 0.0)
            nc.gpsimd.memset(staged[1][:, :, :], 0.0)
            make_identity(nc, idbf)
            make_identity(nc, idf32)

            xv = xw_sb[:, 0:NR * WP].rearrange("p (r w) -> p r w", w=WP)
            w1v = xw_sb[0:3, NR * WP:NR * WP + 256]
            cw = win_sb[:, 0:4608].rearrange("p (l t o) -> p l t o",
                                             t=9, o=128)
            w2pv = win_sb[:, 4608:4608 + 864].rearrange(
                "p (j c) -> p j c", c=432)
            b2pv = win_sb[0:1, 5472:5904]
            cb = f32_sb[:, 0:4]
            b1c = f32_sb[:, 4:6]
            shiftv = f32_sb[:, 6:12]

            # warm ACT's vector clock (1 wait per op) so conv relu-copies
            # only ever wait on PE.
            nc.scalar.copy(dummy[0:1, 0:1], xw_sb[0:1, 0:1])
            nc.scalar.copy(dummy[0:1, 1:2], win_sb[0:1, 0:1])
            nc.scalar.copy(dummy[0:1, 2:3], fA[0:1, 0:1, 0:1])
            nc.scalar.copy(dummy[0:1, 3:4], fB[0:1, 0:1, 0:1])

            # ---- conv chain + interleaved im2col ---------------------
            # l: 0:x->fA  1:fA->fB  2:fB->fA  3:fA->fB
            fins = [xv, fA, fB, fA]
            fouts = [fA, fB, fA, fB]

            def compact_rows(r0, r1):
                for n in range(4):
                    nc.sync.dma_start(
                        out=f4c[16 * n:16 * n + 16, r0:r1, :],
                        in_=fB[32 * n:32 * n + 16, r0:r1, :])

            def transpose_rows(rr):
                # r in fT2 coords (f4 row = r+3)
                for r in rr:
                    for hf in range(2):
                        tp = tps.tile([128, 3, 64], bf, tag="tps")
                        for kw in range(3):
                            nc.tensor.transpose(
                                tp[:, kw, :],
                                f4c[:, r + 3, 128 * hf + kw:
                                    128 * hf + kw + 128],
                                idbf[:, :])
                        nc.vector.tensor_copy(
                            fT2h[hf][:, 3 * r * 64:3 * (r + 1) * 64],
                            tp[:, :, :])

            with tc.tile_pool(name="cps", bufs=2, space="PSUM") as cps, \
                 tc.tile_pool(name="tps", bufs=3, space="PSUM") as tps:
                for l in range(4):
                    fin, fout = fins[l], fouts[l]
                    for ch in range(19):
                        r0 = 1 + 2 * ch
                        ps = cps.tile([128, 2, 256], f32, tag="convps")
                        for tap in range(9):
                            kh, kw = tap // 3, tap % 3
                            if l == 0:
                                lhsT = cw[0:12, 0, tap, :]
                                rhs = fin[0:12, r0 + kh - 1:r0 + kh + 1,
                                          kw:kw + 256]
                            else:
                                lhsT = cw[:, l, tap, :]
                                rhs = fin[:, r0 + kh - 1:r0 + kh + 1,
                                          kw:kw + 256]
                            nc.tensor.matmul(
                                ps[:, :, :], lhsT, rhs,
                                start=(tap == 0), stop=(tap == 8))
                        nc.scalar.activation(
                            fout[:, r0:r0 + 2, 1:257], ps[:, :, :],
                            mybir.ActivationFunctionType.Relu,
                            bias=cb[:, l:l + 1], scale=1.0)
                        # layer 3: compact + transpose finished row groups
                        if l == 3:
                            if ch == 7:
                                compact_rows(3, 13)      # f4 rows 3-12
                            elif ch == 9:
                                transpose_rows(range(0, 10))
                            elif ch == 12:
                                compact_rows(13, 23)
                            elif ch == 14:
                                transpose_rows(range(10, 20))
                            elif ch == 17:
                                compact_rows(23, 33)
                    if l == 3:
                        compact_rows(33, 37)
                        transpose_rows(range(20, 34))

            # ---- per-q: h MLP, local weights, einsum, writeback ------
            with tc.tile_pool(name="hps", bufs=2, space="PSUM") as hps, \
                 tc.tile_pool(name="lps", bufs=3, space="PSUM") as lps, \
                 tc.tile_pool(name="wps", bufs=2, space="PSUM") as wps:
                fT2v = [t.rearrange("p (t x) -> p t x", x=64)
                        for t in fT2h]
                for q in range(4):
                    si, sj = q // 2, q % 2
                    for pc in range(8):
                        pos_t = pos_p.tile([3, 1024], bf, tag="pos")
                        nc.scalar.dma_start(
                            pos_t[:, :],
                            post[q, :, pc * 1024:(pc + 1) * 1024])
                        hT = ht_p.tile([128, 2, 1024], bf, tag="ht")
                        for jh in range(2):
                            for hf2 in range(2):
                                hp = hps.tile([128, 512], f32, tag="hps")
                                nc.tensor.matmul(
                                    hp[:, :],
                                    w1v[:, jh * 128:(jh + 1) * 128],
                                    pos_t[:, hf2 * 512:(hf2 + 1) * 512],
                                    start=True, stop=True)
                                nc.scalar.activation(
                                    hT[:, jh, hf2 * 512:(hf2 + 1) * 512],
                                    hp[:, :],
                                    mybir.ActivationFunctionType.Relu,
                                    bias=b1c[:, jh:jh + 1], scale=1.0)
                        for tl in range(8):
                            t = pc * 8 + tl
                            r0, hf = t // 2, t % 2
                            lwp = lps.tile([128, 3, 9, 16], f32,
                                           tag="lwp")
                            for jh in range(2):
                                nc.tensor.matmul(
                                    lwp[:, :, :, :],
                                    hT[:, jh, tl * 128:(tl + 1) * 128],
                                    w2pv[:, jh, :],
                                    start=(jh == 0), stop=False)
                            nc.tensor.matmul(
                                lwp[:, :, :, :], ones_sb[:, :], b2pv,
                                start=False, stop=True)
                            lws = lws_p.tile([128, 3, 9, 16], bf,
                                             tag="lws")
                            nc.scalar.activation(
                                lws[:, :, :, :], lwp[:, :, :, :],
                                mybir.ActivationFunctionType.Copy)
                            # DVE: broadcast products (per c; n bcast on
                            # in1) -- TT APs allow at most 3 free dims
                            scr2 = scr_p.tile([128, 3, 4, 9, 16], bf,
                                              tag="scr2")
                            in0 = fT2v[hf][:, 3 * r0:3 * r0 + 9, :] \
                                .rearrange("p t (n i) -> p t n i", n=4) \
                                .transpose([0, 2, 1, 3])
                            for c in range(3):
                                in1 = lws[:, c, :, :].unsqueeze(1) \
                                    .broadcast_to([128, 4, 9, 16])
                                nc.vector.tensor_tensor(
                                    out=scr2[:, c, :, :, :],
                                    in0=in0, in1=in1, op=mul)
                            # Pool: fold ci halves (144 -> 72 per pair);
                            # merge (c,n) -> 3 free dims for the Pool ISA
                            scrf = scrf_p.tile([128, 12, 9, 8], bf,
                                               tag="scrf")
                            s2m = scr2.rearrange(
                                "p c n t i -> p (c n) t i")
                            nc.gpsimd.tensor_tensor(
                                out=scrf[:, :, :, :],
                                in0=s2m[:, :, :, 0:8],
                                in1=s2m[:, :, :, 8:16],
                                op=mybir.AluOpType.add)
                            scrfv = scrf.rearrange(
                                "p (c n) t i -> p c n t i", c=3)
                            # DVE: segmented reduces
                            oqv = outq[q].rearrange(
                                "p (n c t) -> p c n t", c=3, t=64)
                            nc.vector.tensor_reduce(
                                out=oqv[:, 0:DVE_C, :, t],
                                in_=scrfv[:, 0:DVE_C, :, :, :],
                                axis=mybir.AxisListType.XY,
                                op=mybir.AluOpType.add)
                            if EXTRA_N:
                                nc.vector.tensor_reduce(
                                    out=oqv[:, DVE_C:DVE_C + 1,
                                            0:EXTRA_N, t],
                                    in_=scrfv[:, DVE_C, 0:EXTRA_N, :, :]
                                    .unsqueeze(1),
                                    axis=mybir.AxisListType.XY,
                                    op=mybir.AluOpType.add)
                            # ACT: accum-copies for the rest
                            for n in range(EXTRA_N, 4):
                                c = DVE_C
                                aj = aj_p.tile([128, 9, 8], bf,
                                               tag="aj")
                                nc.scalar.activation(
                                    aj[:, :, :],
                                    scrfv[:, c, n, :, :],
                                    mybir.ActivationFunctionType.Copy,
                                    accum_out=outq[q][
                                        :, (n * 3 + c) * 64 + t:
                                        (n * 3 + c) * 64 + t + 1])
                    # writeback: transpose + sj-interleave (+mean shift)
                    for j in range(6):
                        tq = wps.tile([128, 128], f32, tag="wps")
                        nc.tensor.transpose(
                            tq[:, :], outq[q][:, 128 * j:128 * (j + 1)],
                            idf32[:, :])
                        nc.scalar.activation(
                            staged[si].rearrange(
                                "p j (w s) -> p j w s", s=2)[:, j, :, sj],
                            tq[:, :],
                            mybir.ActivationFunctionType.Identity,
                            bias=shiftv[:, j:j + 1], scale=1.0)
                    if sj == 1:
                        # src partitions walk (a=nci_lo, r, h)-major then w;
                        # dst dims [a, r, h, w] match that element order.
                        dstv = outd.rearrange(
                            "n c (r s) (h w) -> (n c) s r h w",
                            s=2, h=2)
                        for j in range(6):
                            nc.sync.dma_start(
                                out=dstv[2 * j:2 * j + 2, si, :, :, :],
                                in_=staged[si][:, j, :])
    _legalize_waits(nc)
    return nc


def _get_nc():
    global _NC
    if _NC is None:
        _NC = _build_program()
    return _NC


def _prep_inputs(x, pos_mat, c0w, c0b, c1w, c1b, c2w, c2b, c3w, c3b,
                 w1, b1, w2, b2):
    """Host-side packing of per-core input dicts."""
    x = np.asarray(x, np.float32)
    pos = np.asarray(pos_mat, np.float32).reshape(-1, 3)

    # block-diagonal conv weights cwB[p, l, tap, 32n+co]
    cwB = np.zeros((128, 4, 9, 128), np.float32)
    cbp = np.zeros((128, 4), np.float32)
    for l, (wl, bl) in enumerate(((c0w, c0b), (c1w, c1b),
                                  (c2w, c2b), (c3w, c3b))):
        wl = np.asarray(wl, np.float32)          # (co, ci, 3, 3)
        K = wl.shape[1]
        t = wl.transpose(1, 2, 3, 0).reshape(K, 9, 16)   # (ci, tap, co)
        for n in range(4):
            if l == 0:
                cwB[3 * n:3 * n + K, l, :, 32 * n:32 * n + 16] = t
            else:
                cwB[32 * n:32 * n + K, l, :, 32 * n:32 * n + 16] = t
            cbp[32 * n:32 * n + 16, l] = np.asarray(bl, np.float32)

    w1 = np.asarray(w1, np.float32)              # (3, 256)
    b1p = np.asarray(b1, np.float32).reshape(2, 128).T.copy()  # [j, jh]

    # w2 columns: orig (s=ci*9+tap, c) -> permuted (c, tap, ci)
    w2 = np.asarray(w2, np.float32).reshape(256, 16, 9, 3)     # j,ci,tap,c
    w2pm = w2.transpose(0, 3, 2, 1).reshape(256, 432)          # j,(c,t,ci)
    w2pk = w2pm.reshape(2, 128, 432)                           # [jh,j,432]
    w2pk = np.ascontiguousarray(w2pk.transpose(1, 0, 2))       # [j,jh,432]
    b2 = np.asarray(b2, np.float32).reshape(16, 9, 3)
    b2pk = b2.transpose(2, 1, 0).reshape(432)

    # win pack: [cwB | w2p | b2p]
    winpk = np.zeros((128, WW), np.float32)
    winpk[:, 0:4608] = cwB.reshape(128, 4608)
    winpk[:, 4608:5472] = w2pk.reshape(128, 864)
    winpk[:, 5472:5904] = b2pk[None, :]

    # fin32: [cb | b1c | shift(j)]
    f32pk = np.zeros((128, 12), np.float32)
    f32pk[:, 0:4] = cbp
    f32pk[:, 4:6] = b1p
    for j in range(6):
        for p in range(128):
            nci = 2 * j + (1 if p >= 64 else 0)
            f32pk[p, 6 + j] = RGB_RANGE * RGB_MEAN[nci % 3]

    # pos rows ordered (h, si, w, sj); per-core chunk -> (q, 3, NPIX)
    posr = pos.reshape(Himg, 2, Wimg, 2, 3)

    in_maps = []
    for core in range(NCORES):
        h0 = core * ROWS
        xh = np.zeros((12, NR, WP), np.float32)
        lo, hi = h0 - HALO, h0 + ROWS + HALO
        slo, shi = max(lo, 0), min(hi, Himg)
        for n in range(4):
            xh[3 * n:3 * n + 3, slo - lo:shi - lo, 1:257] = \
                x[n, :, slo:shi, :]
        xpack = np.zeros((12, XW), np.float32)
        xpack[:, :NR * WP] = xh.reshape(12, -1)
        xpack[0:3, NR * WP:] = w1
        pc = posr[h0:h0 + ROWS].transpose(1, 3, 4, 0, 2)  # si,sj,3,h,w
        pc = pc.reshape(4, 3, NPIX)
        in_maps.append({
            "xin": xpack.astype(BF16),
            "win": winpk.astype(BF16),
            "fin32": f32pk,
            "post": np.ascontiguousarray(pc).astype(BF16),
        })
    return in_maps


LAST_RESULTS = None
TRACE = False


def kernel(**inputs):
    global LAST_RESULTS
    nc = _get_nc()
    in_maps = _prep_inputs(**inputs)
    res = run_bass_kernel_spmd(nc, in_maps, core_ids=list(range(NCORES)),
                               trace=TRACE)
    LAST_RESULTS = res
    out = np.concatenate([res.results[i]["out"] for i in range(NCORES)],
                         axis=2)
    return out.astype(np.float32)


# revision 27
# speedup vs baseline: 1.1939x; 1.1922x over previous
"""MetaQuickSR Trainium2 kernel (8-core SPMD, row-sharded), v2.

Sharding: H=256 output-feature rows split 32/core (+4-row conv halo).
Each core: 4-layer CNN (block-diagonal image batching on PE) -> PE-based
im2col row transposes -> bf16 Pos2Weight MLP -> per-pixel locally-
connected einsum split across DVE+Pool -> transpose/interleave writeback
with contiguous output DMAs.  No cross-core communication.
"""

import numpy as np
import ml_dtypes

import concourse.bass as bass
import concourse.mybir as mybir
from concourse.tile import TileContext
from concourse.bass_utils import run_bass_kernel_spmd
from concourse.masks import make_identity

BF16 = ml_dtypes.bfloat16

NCORES = 8
N, CI, Himg, Wimg, S = 4, 16, 256, 256, 2
ROWS = Himg // NCORES          # 32 output-feature rows per core
HALO = 4
NR = ROWS + 2 * HALO           # 40 buffered rows
WP = Wimg + 2                  # 258 zero-padded width
NPIX = ROWS * Wimg             # 8192 einsum pixels per core
NT = NPIX // 128               # 64 pixel tiles per q plane
RGB_MEAN = (0.4488, 0.4371, 0.404)
RGB_RANGE = 255.0

XW = NR * WP + 256             # dense x (12 parts) + w1 rows 0-2
WW = 4 * 9 * 128 + 2 * 432 + 432   # cwB + w2p + b2p(row0)

# einsum reduction split (after a Pool ci-fold halves the volume):
# DVE seg-reduces c < DVE_C in one op plus n < EXTRA_N of c=DVE_C in a
# second; ACT accum-copies the remaining (c, n) pairs.
DVE_C = 2
EXTRA_N = 2

_NC = None


def _legalize_waits(nc, lim=1):
    """This walrus build accepts only one sync-wait per instruction; move
    surplus waits onto same-engine NoOps inserted just before."""
    cnt = 0
    for f in nc.m.functions:
        for bb in f.blocks:
            new = []
            for inst in bb.instructions:
                si = inst.sync_info
                if si is not None and si.on_wait is not None \
                        and len(si.on_wait) > lim:
                    waits = list(si.on_wait)
                    excess, keep = waits[:-lim], waits[-lim:]
                    for w in excess:
                        cnt += 1
                        nop = mybir.InstNoOp(
                            name=f"I-lw{cnt}", opcode="NoOp",
                            engine=inst.engine, debug=inst.debug,
                            ins=[], outs=[],
                            sync_info=mybir.SyncInfo(on_wait=[w],
                                                     on_update=[]))
                        new.append(nop)
                        nc.inst_map[nop.name] = nop
                    inst.sync_info = mybir.SyncInfo(
                        on_wait=keep, on_update=list(si.on_update or []))
                new.append(inst)
            bb.instructions = new
    return cnt


def _build_program():
    nc = bass.Bass(trn_type="TRN2")
    f32 = mybir.dt.float32
    bf = mybir.dt.bfloat16

    xin = nc.dram_tensor("xin", [12, XW], bf, kind="ExternalInput")
    win = nc.dram_tensor("win", [128, WW], bf, kind="ExternalInput")
    fin32 = nc.dram_tensor("fin32", [128, 12], f32, kind="ExternalInput")
    post = nc.dram_tensor("post", [4, 3, NPIX], bf, kind="ExternalInput")
    outd = nc.dram_tensor("out", [4, 3, 2 * ROWS, 2 * Wimg], f32,
                          kind="ExternalOutput")

    mul = mybir.AluOpType.mult

    with TileContext(nc) as tc:
        with (
            tc.tile_pool(name="singles", bufs=1) as singles,
            tc.tile_pool(name="pos_p", bufs=2) as pos_p,
            tc.tile_pool(name="ht_p", bufs=2) as ht_p,
            tc.tile_pool(name="lws_p", bufs=4) as lws_p,
            tc.tile_pool(name="scr_p", bufs=4) as scr_p,
            tc.tile_pool(name="aj_p", bufs=4) as aj_p,
        ):
            # ---- resident tiles --------------------------------------
            xw_sb = singles.tile([12, XW], bf)
            win_sb = singles.tile([128, WW], bf)
            f32_sb = singles.tile([128, 12], f32)
            fA = singles.tile([128, NR, WP], bf)
            fB = singles.tile([128, NR, WP], bf)
            f4c = singles.tile([64, NR, WP], bf)
            # fT2h[hf][pix, (row 34, kw 3, (n,ci) 64)]
            fT2h = [singles.tile([128, 34 * 3 * 64], bf, name=f"fT2h{h}")
                    for h in range(2)]
            outq = [singles.tile([128, 768], f32, name=f"outq{q}")
                    for q in range(4)]
            staged = [singles.tile([128, 6, 256], f32, name=f"stg{s}")
                      for s in range(2)]
            ones_sb = singles.tile([1, 128], bf)
            idbf = singles.tile([64, 64], bf)
            idf32 = singles.tile([128, 128], f32)
            dummy = singles.tile([1, 16], bf)

            nc.sync.dma_start(xw_sb[:, :], xin[:, :])
            nc.scalar.dma_start(win_sb[:, :], win[:, :])
            nc.scalar.dma_start(f32_sb[:, :], fin32[:, :])
            nc.gpsimd.memset(fA[:, :, :], 0.0)
            nc.gpsimd.memset(fB[:, :, :], 0.0)
            nc.gpsimd.memset(ones_sb[:, :], 1.0)
            nc.gpsimd.memset(staged[0][:, :, :], 0.0)
            nc.gpsimd.memset(staged[1][:, :, :], 0.0)
            make_identity(nc, idbf)
            make_identity(nc, idf32)

            xv = xw_sb[:, 0:NR * WP].rearrange("p (r w) -> p r w", w=WP)
            w1v = xw_sb[0:3, NR * WP:NR * WP + 256]
            cw = win_sb[:, 0:4608].rearrange("p (l t o) -> p l t o",
                                             t=9, o=128)
            w2pv = win_sb[:, 4608:4608 + 864].rearrange(
                "p (j c) -> p j c", c=432)
            b2pv = win_sb[0:1, 5472:5904]
            cb = f32_sb[:, 0:4]
            b1c = f32_sb[:, 4:6]
            shiftv = f32_sb[:, 6:12]

            # warm ACT's vector clock (1 wait per op) so conv relu-copies
            # only ever wait on PE.
            nc.scalar.copy(dummy[0:1, 0:1], xw_sb[0:1, 0:1])
            nc.scalar.copy(dummy[0:1, 1:2], win_sb[0:1, 0:1])
            nc.scalar.copy(dummy[0:1, 2:3], fA[0:1, 0:1, 0:1])
            nc.scalar.copy(dummy[0:1, 3:4], fB[0:1, 0:1, 0:1])

            # ---- conv chain + interleaved im2col ---------------------
            # l: 0:x->fA  1:fA->fB  2:fB->fA  3:fA->fB
            fins = [xv, fA, fB, fA]
            fouts = [fA, fB, fA, fB]

            def compact_rows(r0, r1):
                for n in range(4):
                    nc.sync.dma_start(
                        out=f4c[16 * n:16 * n + 16, r0:r1, :],
                        in_=fB[32 * n:32 * n + 16, r0:r1, :])

            def transpose_rows(rr):
                # r in fT2 coords (f4 row = r+3)
                for r in rr:
                    for hf in range(2):
                        tp = tps.tile([128, 3, 64], bf, tag="tps")
                        for kw in range(3):
                            nc.tensor.transpose(
                                tp[:, kw, :],
                                f4c[:, r + 3, 128 * hf + kw:
                                    128 * hf + kw + 128],
                                idbf[:, :])
                        nc.vector.tensor_copy(
                            fT2h[hf][:, 3 * r * 64:3 * (r + 1) * 64],
                            tp[:, :, :])

            with tc.tile_pool(name="cps", bufs=2, space="PSUM") as cps, \
                 tc.tile_pool(name="tps", bufs=3, space="PSUM") as tps:
                for l in range(4):
                    fin, fout = fins[l], fouts[l]
                    for ch in range(19):
                        r0 = 1 + 2 * ch
                        ps = cps.tile([128, 2, 256], f32, tag="convps")
                        for tap in range(9):
                            kh, kw = tap // 3, tap % 3
                            if l == 0:
                                lhsT = cw[0:12, 0, tap, :]
                                rhs = fin[0:12, r0 + kh - 1:r0 + kh + 1,
                                          kw:kw + 256]
                            else:
                                lhsT = cw[:, l, tap, :]
                                rhs = fin[:, r0 + kh - 1:r0 + kh + 1,
                                          kw:kw + 256]
                            nc.tensor.matmul(
                                ps[:, :, :], lhsT, rhs,
                                start=(tap == 0), stop=(tap == 8))
                        nc.scalar.activation(
                            fout[:, r0:r0 + 2, 1:257], ps[:, :, :],
                            mybir.ActivationFunctionType.Relu,
                            bias=cb[:, l:l + 1], scale=1.0)
                        # layer 3: compact + transpose finished row groups
                        if l == 3:
                            if ch == 7:
                                compact_rows(3, 13)      # f4 rows 3-12
                            elif ch == 9:
                                transpose_rows(range(0, 10))
                            elif ch == 12:
                                compact_rows(13, 23)
                            elif ch == 14:
                                transpose_rows(range(10, 20))
                            elif ch == 17:
                                compact_rows(23, 33)
                    if l == 3:
                        compact_rows(33, 37)
                        transpose_rows(range(20, 34))

            # ---- per-q: h MLP, local weights, einsum, writeback ------
            with tc.tile_pool(name="hps", bufs=2, space="PSUM") as hps, \
                 tc.tile_pool(name="lps", bufs=3, space="PSUM") as lps, \
                 tc.tile_pool(name="wps", bufs=2, space="PSUM") as wps:
                fT2v = [t.rearrange("p (t x) -> p t x", x=64)
                        for t in fT2h]
                for q in range(4):
                    si, sj = q // 2, q % 2
                    for pc in range(8):
                        pos_t = pos_p.tile([3, 1024], bf, tag="pos")
                        nc.scalar.dma_start(
                            pos_t[:, :],
                            post[q, :, pc * 1024:(pc + 1) * 1024])
                        hT = ht_p.tile([128, 2, 1024], bf, tag="ht")
                        for jh in range(2):
                            for hf2 in range(2):
                                hp = hps.tile([128, 512], f32, tag="hps")
                                nc.tensor.matmul(
                                    hp[:, :],
                                    w1v[:, jh * 128:(jh + 1) * 128],
                                    pos_t[:, hf2 * 512:(hf2 + 1) * 512],
                                    start=True, stop=True)
                                nc.scalar.activation(
                                    hT[:, jh, hf2 * 512:(hf2 + 1) * 512],
                                    hp[:, :],
                                    mybir.ActivationFunctionType.Relu,
                                    bias=b1c[:, jh:jh + 1], scale=1.0)
                        for tl in range(8):
                            t = pc * 8 + tl
                            r0, hf = t // 2, t % 2
                            lwp = lps.tile([128, 3, 9, 16], f32,
                                           tag="lwp")
                            for jh in range(2):
                                nc.tensor.matmul(
                                    lwp[:, :, :, :],
                                    hT[:, jh, tl * 128:(tl + 1) * 128],
                                    w2pv[:, jh, :],
                                    start=(jh == 0), stop=False)
                            nc.tensor.matmul(
                                lwp[:, :, :, :], ones_sb[:, :], b2pv,
                                start=False, stop=True)
                            lws = lws_p.tile([128, 3, 9, 16], bf,
                                             tag="lws")
                            nc.scalar.activation(
                                lws[:, :, :, :], lwp[:, :, :, :],
                                mybir.ActivationFunctionType.Copy)
                            # DVE: STT+accum for 9 pairs (c<2, + c2n0)
                            for c in range(3):
                                for n in range(4):
                                    if c == 2 and n > 0:
                                        continue
                                    scr = scr_p.tile([128, 9, 16], bf,
                                                     tag="scr")
                                    nc.vector.scalar_tensor_tensor(
                                        out=scr[:, :, :],
                                        in0=fT2v[hf][:,
                                                     3 * r0:3 * r0 + 9,
                                                     16 * n:16 * n + 16],
                                        scalar=1.0,
                                        in1=lws[:, c, :, :],
                                        op0=mul, op1=mul,
                                        accum_out=outq[q][
                                            :, (n * 3 + c) * 64 + t:
                                            (n * 3 + c) * 64 + t + 1])
                            # DVE: one product for (c=2, n=1..3) pairs
                            scr2 = scr_p.tile([128, 3, 9, 16], bf,
                                              tag="scr2")
                            in0 = fT2v[hf][:, 3 * r0:3 * r0 + 9, :] \
                                .rearrange("p t (n i) -> p t n i", n=4) \
                                .transpose([0, 2, 1, 3])
                            nc.vector.tensor_tensor(
                                out=scr2[:, :, :, :],
                                in0=in0[:, 1:4, :, :],
                                in1=lws[:, 2, :, :].unsqueeze(1)
                                .broadcast_to([128, 3, 9, 16]), op=mul)
                            # ACT: accum-copies for those 3 pairs
                            for n in range(1, 4):
                                aj = aj_p.tile([128, 9, 16], bf,
                                               tag="aj")
                                nc.scalar.activation(
                                    aj[:, :, :],
                                    scr2[:, n - 1, :, :],
                                    mybir.ActivationFunctionType.Copy,
                                    accum_out=outq[q][
                                        :, (n * 3 + 2) * 64 + t:
                                        (n * 3 + 2) * 64 + t + 1])
                    # writeback: transpose + sj-interleave (+mean shift)
                    for j in range(6):
                        tq = wps.tile([128, 128], f32, tag="wps")
                        nc.tensor.transpose(
                            tq[:, :], outq[q][:, 128 * j:128 * (j + 1)],
                            idf32[:, :])
                        nc.scalar.activation(
                            staged[si].rearrange(
                                "p j (w s) -> p j w s", s=2)[:, j, :, sj],
                            tq[:, :],
                            mybir.ActivationFunctionType.Identity,
                            bias=shiftv[:, j:j + 1], scale=1.0)
                    if sj == 1:
                        # src partitions walk (a=nci_lo, r, h)-major then w;
                        # dst dims [a, r, h, w] match that element order.
                        dstv = outd.rearrange(
                            "n c (r s) (h w) -> (n c) s r h w",
                            s=2, h=2)
                        for j in range(6):
                            nc.sync.dma_start(
                                out=dstv[2 * j:2 * j + 2, si, :, :, :],
                                in_=staged[si][:, j, :])
    _legalize_waits(nc)
    return nc


def _get_nc():
    global _NC
    if _NC is None:
        _NC = _build_program()
    return _NC


def _prep_inputs(x, pos_mat, c0w, c0b, c1w, c1b, c2w, c2b, c3w, c3b,
                 w1, b1, w2, b2):
    """Host-side packing of per-core input dicts."""
    x = np.asarray(x, np.float32)
    pos = np.asarray(pos_mat, np.float32).reshape(-1, 3)

    # block-diagonal conv weights cwB[p, l, tap, 32n+co]
    cwB = np.zeros((128, 4, 9, 128), np.float32)
    cbp = np.zeros((128, 4), np.float32)
    for l, (wl, bl) in enumerate(((c0w, c0b), (c1w, c1b),
                                  (c2w, c2b), (c3w, c3b))):
        wl = np.asarray(wl, np.float32)          # (co, ci, 3, 3)
        K = wl.shape[1]
        t = wl.transpose(1, 2, 3, 0).reshape(K, 9, 16)   # (ci, tap, co)
        for n in range(4):
            if l == 0:
                cwB[3 * n:3 * n + K, l, :, 32 * n:32 * n + 16] = t
            else:
                cwB[32 * n:32 * n + K, l, :, 32 * n:32 * n + 16] = t
            cbp[32 * n:32 * n + 16, l] = np.asarray(bl, np.float32)

    w1 = np.asarray(w1, np.float32)              # (3, 256)
    b1p = np.asarray(b1, np.float32).reshape(2, 128).T.copy()  # [j, jh]

    # w2 columns: orig (s=ci*9+tap, c) -> permuted (c, tap, ci)
    w2 = np.asarray(w2, np.float32).reshape(256, 16, 9, 3)     # j,ci,tap,c
    w2pm = w2.transpose(0, 3, 2, 1).reshape(256, 432)          # j,(c,t,ci)
    w2pk = w2pm.reshape(2, 128, 432)                           # [jh,j,432]
    w2pk = np.ascontiguousarray(w2pk.transpose(1, 0, 2))       # [j,jh,432]
    b2 = np.asarray(b2, np.float32).reshape(16, 9, 3)
    b2pk = b2.transpose(2, 1, 0).reshape(432)

    # win pack: [cwB | w2p | b2p]
    winpk = np.zeros((128, WW), np.float32)
    winpk[:, 0:4608] = cwB.reshape(128, 4608)
    winpk[:, 4608:5472] = w2pk.reshape(128, 864)
    winpk[:, 5472:5904] = b2pk[None, :]

    # fin32: [cb | b1c | shift(j)]
    f32pk = np.zeros((128, 12), np.float32)
    f32pk[:, 0:4] = cbp
    f32pk[:, 4:6] = b1p
    for j in range(6):
        for p in range(128):
            nci = 2 * j + (1 if p >= 64 else 0)
            f32pk[p, 6 + j] = RGB_RANGE * RGB_MEAN[nci % 3]

    # pos rows ordered (h, si, w, sj); per-core chunk -> (q, 3, NPIX)
    posr = pos.reshape(Himg, 2, Wimg, 2, 3)

    in_maps = []
    for core in range(NCORES):
        h0 = core * ROWS
        xh = np.zeros((12, NR, WP), np.float32)
        lo, hi = h0 - HALO, h0 + ROWS + HALO
        slo, shi = max(lo, 0), min(hi, Himg)
        for n in range(4):
            xh[3 * n:3 * n + 3, slo - lo:shi - lo, 1:257] = \
                x[n, :, slo:shi, :]
        xpack = np.zeros((12, XW), np.float32)
        xpack[:, :NR * WP] = xh.reshape(12, -1)
        xpack[0:3, NR * WP:] = w1
        pc = posr[h0:h0 + ROWS].transpose(1, 3, 4, 0, 2)  # si,sj,3,h,w
        pc = pc.reshape(4, 3, NPIX)
        in_maps.append({
            "xin": xpack.astype(BF16),
            "win": winpk.astype(BF16),
            "fin32": f32pk,
            "post": np.ascontiguousarray(pc).astype(BF16),
        })
    return in_maps


LAST_RESULTS = None
TRACE = False


def kernel(**inputs):
    global LAST_RESULTS
    nc = _get_nc()
    in_maps = _prep_inputs(**inputs)
    res = run_bass_kernel_spmd(nc, in_maps, core_ids=list(range(NCORES)),
                               trace=TRACE)
    LAST_RESULTS = res
    out = np.concatenate([res.results[i]["out"] for i in range(NCORES)],
                         axis=2)
    return out.astype(np.float32)


# revision 29
# speedup vs baseline: 1.2364x; 1.0356x over previous
"""MetaQuickSR Trainium2 kernel (8-core SPMD, row-sharded), v2.

Sharding: H=256 output-feature rows split 32/core (+4-row conv halo).
Each core: 4-layer CNN (block-diagonal image batching on PE) -> PE-based
im2col row transposes -> bf16 Pos2Weight MLP -> per-pixel locally-
connected einsum split across DVE+Pool -> transpose/interleave writeback
with contiguous output DMAs.  No cross-core communication.
"""

import numpy as np
import ml_dtypes

import concourse.bass as bass
import concourse.mybir as mybir
from concourse.tile import TileContext
from concourse.bass_utils import run_bass_kernel_spmd
from concourse.masks import make_identity

BF16 = ml_dtypes.bfloat16

NCORES = 8
N, CI, Himg, Wimg, S = 4, 16, 256, 256, 2
ROWS = Himg // NCORES          # 32 output-feature rows per core
HALO = 4
NR = ROWS + 2 * HALO           # 40 buffered rows
WP = Wimg + 2                  # 258 zero-padded width
NPIX = ROWS * Wimg             # 8192 einsum pixels per core
NT = NPIX // 128               # 64 pixel tiles per q plane
RGB_MEAN = (0.4488, 0.4371, 0.404)
RGB_RANGE = 255.0

XW = NR * WP + 256             # dense x (12 parts) + w1 rows 0-2
WW = 4 * 9 * 128 + 2 * 432 + 432   # cwB + w2p + b2p(row0)

# einsum reduction split (after a Pool ci-fold halves the volume):
# DVE seg-reduces c < DVE_C in one op plus n < EXTRA_N of c=DVE_C in a
# second; ACT accum-copies the remaining (c, n) pairs.
DVE_C = 2
EXTRA_N = 2

_NC = None


def _legalize_waits(nc, lim=1):
    """This walrus build accepts only one sync-wait per instruction; move
    surplus waits onto same-engine NoOps inserted just before."""
    cnt = 0
    for f in nc.m.functions:
        for bb in f.blocks:
            new = []
            for inst in bb.instructions:
                si = inst.sync_info
                if si is not None and si.on_wait is not None \
                        and len(si.on_wait) > lim:
                    waits = list(si.on_wait)
                    excess, keep = waits[:-lim], waits[-lim:]
                    for w in excess:
                        cnt += 1
                        nop = mybir.InstNoOp(
                            name=f"I-lw{cnt}", opcode="NoOp",
                            engine=inst.engine, debug=inst.debug,
                            ins=[], outs=[],
                            sync_info=mybir.SyncInfo(on_wait=[w],
                                                     on_update=[]))
                        new.append(nop)
                        nc.inst_map[nop.name] = nop
                    inst.sync_info = mybir.SyncInfo(
                        on_wait=keep, on_update=list(si.on_update or []))
                new.append(inst)
            bb.instructions = new
    return cnt


def _build_program():
    nc = bass.Bass(trn_type="TRN2")
    f32 = mybir.dt.float32
    bf = mybir.dt.bfloat16

    xin = nc.dram_tensor("xin", [12, XW], bf, kind="ExternalInput")
    win = nc.dram_tensor("win", [128, WW], bf, kind="ExternalInput")
    fin32 = nc.dram_tensor("fin32", [128, 12], f32, kind="ExternalInput")
    post = nc.dram_tensor("post", [4, 3, NPIX], bf, kind="ExternalInput")
    outd = nc.dram_tensor("out", [4, 3, 2 * ROWS, 2 * Wimg], f32,
                          kind="ExternalOutput")

    mul = mybir.AluOpType.mult

    with TileContext(nc) as tc:
        with (
            tc.tile_pool(name="singles", bufs=1) as singles,
            tc.tile_pool(name="pos_p", bufs=2) as pos_p,
            tc.tile_pool(name="ht_p", bufs=2) as ht_p,
            tc.tile_pool(name="lws_p", bufs=4) as lws_p,
            tc.tile_pool(name="scr_p", bufs=4) as scr_p,
            tc.tile_pool(name="aj_p", bufs=4) as aj_p,
        ):
            # ---- resident tiles --------------------------------------
            xw_sb = singles.tile([12, XW], bf)
            win_sb = singles.tile([128, WW], bf)
            f32_sb = singles.tile([128, 12], f32)
            fA = singles.tile([128, NR, WP], bf)
            fB = singles.tile([128, NR, WP], bf)
            f4c = singles.tile([64, NR, WP], bf)
            # fT2h[hf][pix, (row 34, kw 3, (n,ci) 64)]
            fT2h = [singles.tile([128, 34 * 3 * 64], bf, name=f"fT2h{h}")
                    for h in range(2)]
            outq = [singles.tile([128, 768], f32, name=f"outq{q}")
                    for q in range(4)]
            staged = [singles.tile([128, 6, 256], f32, name=f"stg{s}")
                      for s in range(2)]
            ones_sb = singles.tile([1, 128], bf)
            idbf = singles.tile([64, 64], bf)
            idf32 = singles.tile([128, 128], f32)
            dummy = singles.tile([1, 16], bf)

            nc.sync.dma_start(xw_sb[:, :], xin[:, :])
            nc.scalar.dma_start(win_sb[:, :], win[:, :])
            nc.scalar.dma_start(f32_sb[:, :], fin32[:, :])
            nc.gpsimd.memset(fA[:, :, :], 0.0)
            nc.gpsimd.memset(fB[:, :, :], 0.0)
            nc.gpsimd.memset(ones_sb[:, :], 1.0)
            nc.gpsimd.memset(staged[0][:, :, :], 0.0)
            nc.gpsimd.memset(staged[1][:, :, :], 0.0)
            make_identity(nc, idbf)
            make_identity(nc, idf32)

            xv = xw_sb[:, 0:NR * WP].rearrange("p (r w) -> p r w", w=WP)
            w1v = xw_sb[0:3, NR * WP:NR * WP + 256]
            cw = win_sb[:, 0:4608].rearrange("p (l t o) -> p l t o",
                                             t=9, o=128)
            w2pv = win_sb[:, 4608:4608 + 864].rearrange(
                "p (j c) -> p j c", c=432)
            b2pv = win_sb[0:1, 5472:5904]
            cb = f32_sb[:, 0:4]
            b1c = f32_sb[:, 4:6]
            shiftv = f32_sb[:, 6:12]

            # warm ACT's vector clock (1 wait per op) so conv relu-copies
            # only ever wait on PE.
            nc.scalar.copy(dummy[0:1, 0:1], xw_sb[0:1, 0:1])
            nc.scalar.copy(dummy[0:1, 1:2], win_sb[0:1, 0:1])
            nc.scalar.copy(dummy[0:1, 2:3], fA[0:1, 0:1, 0:1])
            nc.scalar.copy(dummy[0:1, 3:4], fB[0:1, 0:1, 0:1])

            # ---- wavefront conv + interleaved im2col + einsum --------
            # l: 0:x->fA  1:fA->fB  2:fB->fA  3:fA->fB
            fins = [xv, fA, fB, fA]
            fouts = [fA, fB, fA, fB]

            def compact_rows(r0, r1):
                for n in range(4):
                    nc.sync.dma_start(
                        out=f4c[16 * n:16 * n + 16, r0:r1, :],
                        in_=fB[32 * n:32 * n + 16, r0:r1, :])

            def transpose_rows(rr):
                # r in fT2 coords (f4 row = r+3)
                for r in rr:
                    for hf in range(2):
                        tpf = xps.tile([128, 128], f32, tag="tps",
                                       name="tpf")
                        tp = tpf.bitcast(bf)
                        for kw in range(3):
                            nc.tensor.transpose(
                                tp[:, 64 * kw:64 * (kw + 1)],
                                f4c[:, r + 3, 128 * hf + kw:
                                    128 * hf + kw + 128],
                                idbf[:, :])
                        nc.vector.tensor_copy(
                            fT2h[hf][:, 3 * r * 64:3 * (r + 1) * 64],
                            tp[:, 0:192])

            def conv_chunk(l, ch):
                fin, fout = fins[l], fouts[l]
                r0 = 1 + 2 * ch
                ps = cps.tile([128, 2, 256], f32, tag="convps")
                for tap in range(9):
                    kh, kw = tap // 3, tap % 3
                    if l == 0:
                        lhsT = cw[0:12, 0, tap, :]
                        rhs = fin[0:12, r0 + kh - 1:r0 + kh + 1,
                                  kw:kw + 256]
                    else:
                        lhsT = cw[:, l, tap, :]
                        rhs = fin[:, r0 + kh - 1:r0 + kh + 1,
                                  kw:kw + 256]
                    nc.tensor.matmul(
                        ps[:, :, :], lhsT, rhs,
                        start=(tap == 0), stop=(tap == 8))
                nc.scalar.activation(
                    fout[:, r0:r0 + 2, 1:257], ps[:, :, :],
                    mybir.ActivationFunctionType.Relu,
                    bias=cb[:, l:l + 1], scale=1.0)

            def einsum_pc(q, pc):
                pos_t = pos_p.tile([3, 1024], bf, tag="pos")
                nc.scalar.dma_start(
                    pos_t[:, :],
                    post[q, :, pc * 1024:(pc + 1) * 1024])
                hT = ht_p.tile([128, 2, 1024], bf, tag="ht")
                for jh in range(2):
                    for hf2 in range(2):
                        hp = hps.tile([128, 512], f32, tag="hps")
                        nc.tensor.matmul(
                            hp[:, :],
                            w1v[:, jh * 128:(jh + 1) * 128],
                            pos_t[:, hf2 * 512:(hf2 + 1) * 512],
                            start=True, stop=True)
                        nc.scalar.activation(
                            hT[:, jh, hf2 * 512:(hf2 + 1) * 512],
                            hp[:, :],
                            mybir.ActivationFunctionType.Relu,
                            bias=b1c[:, jh:jh + 1], scale=1.0)
                for tl in range(8):
                    t = pc * 8 + tl
                    r0, hf = t // 2, t % 2
                    lwp = lps.tile([128, 3, 9, 16], f32, tag="lwp")
                    for jh in range(2):
                        nc.tensor.matmul(
                            lwp[:, :, :, :],
                            hT[:, jh, tl * 128:(tl + 1) * 128],
                            w2pv[:, jh, :],
                            start=(jh == 0), stop=False)
                    nc.tensor.matmul(
                        lwp[:, :, :, :], ones_sb[:, :], b2pv,
                        start=False, stop=True)
                    lws = lws_p.tile([128, 3, 9, 16], bf, tag="lws")
                    nc.scalar.activation(
                        lws[:, :, :, :], lwp[:, :, :, :],
                        mybir.ActivationFunctionType.Copy)
                    # DVE: STT+accum for 9 pairs (c<2, + c2n0)
                    for c in range(3):
                        for n in range(4):
                            if c == 2 and n > 0:
                                continue
                            scr = scr_p.tile([128, 9, 16], bf,
                                             tag="scr")
                            nc.vector.scalar_tensor_tensor(
                                out=scr[:, :, :],
                                in0=fT2v[hf][:, 3 * r0:3 * r0 + 9,
                                             16 * n:16 * n + 16],
                                scalar=1.0,
                                in1=lws[:, c, :, :],
                                op0=mul, op1=mul,
                                accum_out=outq[q][
                                    :, (n * 3 + c) * 64 + t:
                                    (n * 3 + c) * 64 + t + 1])
                    # DVE: one product for (c=2, n=1..3) pairs
                    scr2 = scr_p.tile([128, 3, 9, 16], bf,
                                      tag="scr2")
                    in0 = fT2v[hf][:, 3 * r0:3 * r0 + 9, :] \
                        .rearrange("p t (n i) -> p t n i", n=4) \
                        .transpose([0, 2, 1, 3])
                    nc.vector.tensor_tensor(
                        out=scr2[:, :, :, :],
                        in0=in0[:, 1:4, :, :],
                        in1=lws[:, 2, :, :].unsqueeze(1)
                        .broadcast_to([128, 3, 9, 16]), op=mul)
                    # ACT: accum-copies for those 3 pairs
                    for n in range(1, 4):
                        aj = aj_p.tile([128, 9, 16], bf, tag="aj")
                        nc.scalar.activation(
                            aj[:, :, :],
                            scr2[:, n - 1, :, :],
                            mybir.ActivationFunctionType.Copy,
                            accum_out=outq[q][
                                :, (n * 3 + 2) * 64 + t:
                                (n * 3 + 2) * 64 + t + 1])

            def writeback(q):
                si, sj = q // 2, q % 2
                for j in range(6):
                    tq = xps.tile([128, 128], f32, tag="tps")
                    nc.tensor.transpose(
                        tq[:, :], outq[q][:, 128 * j:128 * (j + 1)],
                        idf32[:, :])
                    nc.scalar.activation(
                        staged[si].rearrange(
                            "p j (w s) -> p j w s", s=2)[:, j, :, sj],
                        tq[:, :],
                        mybir.ActivationFunctionType.Identity,
                        bias=shiftv[:, j:j + 1], scale=1.0)
                if sj == 1:
                    dstv = outd.rearrange(
                        "n c (r s) (h w) -> (n c) s r h w", s=2, h=2)
                    for j in range(6):
                        nc.sync.dma_start(
                            out=dstv[2 * j:2 * j + 2, si, :, :, :],
                            in_=staged[si][:, j, :])

            fT2v = [tt.rearrange("p (t x) -> p t x", x=64)
                    for tt in fT2h]
            # post-step actions: (after wavefront step s) -> emit list
            post_step = {
                12: [lambda: compact_rows(3, 13)],
                13: [lambda: transpose_rows(range(0, 10))],
                14: [lambda: [einsum_pc(q, 0) for q in range(4)],
                     lambda: [einsum_pc(q, 1) for q in range(4)]],
                15: [lambda: compact_rows(13, 21)],
                16: [lambda: transpose_rows(range(10, 18))],
                17: [lambda: [einsum_pc(q, 2) for q in range(4)],
                     lambda: [einsum_pc(q, 3) for q in range(4)]],
                19: [lambda: compact_rows(21, 29)],
                20: [lambda: transpose_rows(range(18, 26))],
                21: [lambda: [einsum_pc(q, 4) for q in range(4)],
                     lambda: [einsum_pc(q, 5) for q in range(4)]],
                23: [lambda: compact_rows(29, 37)],
            }
            with tc.tile_pool(name="cps", bufs=2, space="PSUM") as cps, \
                 tc.tile_pool(name="xps", bufs=2, space="PSUM") as xps, \
                 tc.tile_pool(name="hps", bufs=2, space="PSUM") as hps, \
                 tc.tile_pool(name="lps", bufs=2, space="PSUM") as lps:
                for step in range(25):
                    for l in range(4):
                        ch = step - 2 * l
                        if 0 <= ch < 19:
                            conv_chunk(l, ch)
                    for fn in post_step.get(step, []):
                        fn()
                transpose_rows(range(26, 34))
                for pc in (6, 7):
                    for q in range(4):
                        einsum_pc(q, pc)
                for q in range(4):
                    writeback(q)
    _legalize_waits(nc)
    return nc


def _get_nc():
    global _NC
    if _NC is None:
        _NC = _build_program()
    return _NC


def _prep_inputs(x, pos_mat, c0w, c0b, c1w, c1b, c2w, c2b, c3w, c3b,
                 w1, b1, w2, b2):
    """Host-side packing of per-core input dicts."""
    x = np.asarray(x, np.float32)
    pos = np.asarray(pos_mat, np.float32).reshape(-1, 3)

    # block-diagonal conv weights cwB[p, l, tap, 32n+co]
    cwB = np.zeros((128, 4, 9, 128), np.float32)
    cbp = np.zeros((128, 4), np.float32)
    for l, (wl, bl) in enumerate(((c0w, c0b), (c1w, c1b),
                                  (c2w, c2b), (c3w, c3b))):
        wl = np.asarray(wl, np.float32)          # (co, ci, 3, 3)
        K = wl.shape[1]
        t = wl.transpose(1, 2, 3, 0).reshape(K, 9, 16)   # (ci, tap, co)
        for n in range(4):
            if l == 0:
                cwB[3 * n:3 * n + K, l, :, 32 * n:32 * n + 16] = t
            else:
                cwB[32 * n:32 * n + K, l, :, 32 * n:32 * n + 16] = t
            cbp[32 * n:32 * n + 16, l] = np.asarray(bl, np.float32)

    w1 = np.asarray(w1, np.float32)              # (3, 256)
    b1p = np.asarray(b1, np.float32).reshape(2, 128).T.copy()  # [j, jh]

    # w2 columns: orig (s=ci*9+tap, c) -> permuted (c, tap, ci)
    w2 = np.asarray(w2, np.float32).reshape(256, 16, 9, 3)     # j,ci,tap,c
    w2pm = w2.transpose(0, 3, 2, 1).reshape(256, 432)          # j,(c,t,ci)
    w2pk = w2pm.reshape(2, 128, 432)                           # [jh,j,432]
    w2pk = np.ascontiguousarray(w2pk.transpose(1, 0, 2))       # [j,jh,432]
    b2 = np.asarray(b2, np.float32).reshape(16, 9, 3)
    b2pk = b2.transpose(2, 1, 0).reshape(432)

    # win pack: [cwB | w2p | b2p]
    winpk = np.zeros((128, WW), np.float32)
    winpk[:, 0:4608] = cwB.reshape(128, 4608)
    winpk[:, 4608:5472] = w2pk.reshape(128, 864)
    winpk[:, 5472:5904] = b2pk[None, :]

    # fin32: [cb | b1c | shift(j)]
    f32pk = np.zeros((128, 12), np.float32)
    f32pk[:, 0:4] = cbp
    f32pk[:, 4:6] = b1p
    for j in range(6):
        for p in range(128):
            nci = 2 * j + (1 if p >= 64 else 0)
            f32pk[p, 6 + j] = RGB_RANGE * RGB_MEAN[nci % 3]

    # pos rows ordered (h, si, w, sj); per-core chunk -> (q, 3, NPIX)
    posr = pos.reshape(Himg, 2, Wimg, 2, 3)

    in_maps = []
    for core in range(NCORES):
        h0 = core * ROWS
        xh = np.zeros((12, NR, WP), np.float32)
        lo, hi = h0 - HALO, h0 + ROWS + HALO
        slo, shi = max(lo, 0), min(hi, Himg)
        for n in range(4):
            xh[3 * n:3 * n + 3, slo - lo:shi - lo, 1:257] = \
                x[n, :, slo:shi, :]
        xpack = np.zeros((12, XW), np.float32)
        xpack[:, :NR * WP] = xh.reshape(12, -1)
        xpack[0:3, NR * WP:] = w1
        pc = posr[h0:h0 + ROWS].transpose(1, 3, 4, 0, 2)  # si,sj,3,h,w
        pc = pc.reshape(4, 3, NPIX)
        in_maps.append({
            "xin": xpack.astype(BF16),
            "win": winpk.astype(BF16),
            "fin32": f32pk,
            "post": np.ascontiguousarray(pc).astype(BF16),
        })
    return in_maps


LAST_RESULTS = None
TRACE = False


def kernel(**inputs):
    global LAST_RESULTS
    nc = _get_nc()
    in_maps = _prep_inputs(**inputs)
    res = run_bass_kernel_spmd(nc, in_maps, core_ids=list(range(NCORES)),
                               trace=TRACE)
    LAST_RESULTS = res
    out = np.concatenate([res.results[i]["out"] for i in range(NCORES)],
                         axis=2)
    return out.astype(np.float32)


# revision 31
# speedup vs baseline: 1.3104x; 1.0598x over previous
"""MetaQuickSR Trainium2 kernel (8-core SPMD, row-sharded), v2.

Sharding: H=256 output-feature rows split 32/core (+4-row conv halo).
Each core: 4-layer CNN (block-diagonal image batching on PE) -> PE-based
im2col row transposes -> bf16 Pos2Weight MLP -> per-pixel locally-
connected einsum split across DVE+Pool -> transpose/interleave writeback
with contiguous output DMAs.  No cross-core communication.
"""

import numpy as np
import ml_dtypes

import concourse.bass as bass
import concourse.mybir as mybir
from concourse.tile import TileContext
from concourse.bass_utils import run_bass_kernel_spmd
from concourse.masks import make_identity

BF16 = ml_dtypes.bfloat16

NCORES = 8
N, CI, Himg, Wimg, S = 4, 16, 256, 256, 2
ROWS = Himg // NCORES          # 32 output-feature rows per core
HALO = 4
NR = ROWS + 2 * HALO           # 40 buffered rows
WP = Wimg + 2                  # 258 zero-padded width
NPIX = ROWS * Wimg             # 8192 einsum pixels per core
NT = NPIX // 128               # 64 pixel tiles per q plane
RGB_MEAN = (0.4488, 0.4371, 0.404)
RGB_RANGE = 255.0

XW = NR * WP + 256             # dense x (12 parts) + w1 rows 0-2
WW = 4 * 9 * 128 + 2 * 432 + 432   # cwB + w2p + b2p(row0)

# einsum reduction split (after a Pool ci-fold halves the volume):
# DVE seg-reduces c < DVE_C in one op plus n < EXTRA_N of c=DVE_C in a
# second; ACT accum-copies the remaining (c, n) pairs.
DVE_C = 2
EXTRA_N = 2

_NC = None


def _legalize_waits(nc, lim=1):
    """This walrus build accepts only one sync-wait per instruction; move
    surplus waits onto same-engine NoOps inserted just before."""
    cnt = 0
    for f in nc.m.functions:
        for bb in f.blocks:
            new = []
            for inst in bb.instructions:
                si = inst.sync_info
                if si is not None and si.on_wait is not None \
                        and len(si.on_wait) > lim:
                    waits = list(si.on_wait)
                    excess, keep = waits[:-lim], waits[-lim:]
                    for w in excess:
                        cnt += 1
                        nop = mybir.InstNoOp(
                            name=f"I-lw{cnt}", opcode="NoOp",
                            engine=inst.engine, debug=inst.debug,
                            ins=[], outs=[],
                            sync_info=mybir.SyncInfo(on_wait=[w],
                                                     on_update=[]))
                        new.append(nop)
                        nc.inst_map[nop.name] = nop
                    inst.sync_info = mybir.SyncInfo(
                        on_wait=keep, on_update=list(si.on_update or []))
                new.append(inst)
            bb.instructions = new
    return cnt


def _build_program():
    nc = bass.Bass(trn_type="TRN2")
    f32 = mybir.dt.float32
    bf = mybir.dt.bfloat16

    xin = nc.dram_tensor("xin", [12, XW], bf, kind="ExternalInput")
    win = nc.dram_tensor("win", [128, WW], bf, kind="ExternalInput")
    fin32 = nc.dram_tensor("fin32", [128, 12], f32, kind="ExternalInput")
    post = nc.dram_tensor("post", [4, 3, NPIX], bf, kind="ExternalInput")
    outd = nc.dram_tensor("out", [4, 3, 2 * ROWS, 2 * Wimg], f32,
                          kind="ExternalOutput")

    mul = mybir.AluOpType.mult

    with TileContext(nc) as tc:
        with (
            tc.tile_pool(name="singles", bufs=1) as singles,
            tc.tile_pool(name="pos_p", bufs=4) as pos_p,
            tc.tile_pool(name="ht_p", bufs=3) as ht_p,
            tc.tile_pool(name="lws_p", bufs=6) as lws_p,
            tc.tile_pool(name="scr_p", bufs=6) as scr_p,
            tc.tile_pool(name="aj_p", bufs=6) as aj_p,
        ):
            # ---- resident tiles --------------------------------------
            xw_sb = singles.tile([12, XW], bf)
            win_sb = singles.tile([128, WW], bf)
            f32_sb = singles.tile([128, 12], f32)
            fA = singles.tile([128, NR, WP], bf)
            fB = singles.tile([128, NR, WP], bf)
            f4c = singles.tile([64, NR, WP], bf)
            # fT2h[hf][pix, (row 34, kw 3, (n,ci) 64)]
            fT2h = [singles.tile([128, 34 * 3 * 64], bf, name=f"fT2h{h}")
                    for h in range(2)]
            outq = [singles.tile([128, 768], f32, name=f"outq{q}")
                    for q in range(4)]
            staged = [singles.tile([128, 6, 256], f32, name=f"stg{s}")
                      for s in range(2)]
            ones_sb = singles.tile([1, 128], bf)
            idbf = singles.tile([64, 64], bf)
            idf32 = singles.tile([128, 128], f32)
            dummy = singles.tile([1, 16], bf)

            nc.sync.dma_start(xw_sb[:, :], xin[:, :])
            nc.scalar.dma_start(win_sb[:, :], win[:, :])
            nc.scalar.dma_start(f32_sb[:, :], fin32[:, :])
            nc.gpsimd.memset(fA[:, :, :], 0.0)
            nc.gpsimd.memset(fB[:, :, :], 0.0)
            nc.gpsimd.memset(ones_sb[:, :], 1.0)
            nc.gpsimd.memset(staged[0][:, :, :], 0.0)
            nc.gpsimd.memset(staged[1][:, :, :], 0.0)
            make_identity(nc, idbf)
            make_identity(nc, idf32)

            xv = xw_sb[:, 0:NR * WP].rearrange("p (r w) -> p r w", w=WP)
            w1v = xw_sb[0:3, NR * WP:NR * WP + 256]
            cw = win_sb[:, 0:4608].rearrange("p (l t o) -> p l t o",
                                             t=9, o=128)
            w2pv = win_sb[:, 4608:4608 + 864].rearrange(
                "p (j c) -> p j c", c=432)
            b2pv = win_sb[0:1, 5472:5904]
            cb = f32_sb[:, 0:4]
            b1c = f32_sb[:, 4:6]
            shiftv = f32_sb[:, 6:12]

            # warm ACT's vector clock (1 wait per op) so conv relu-copies
            # only ever wait on PE.
            nc.scalar.copy(dummy[0:1, 0:1], xw_sb[0:1, 0:1])
            nc.scalar.copy(dummy[0:1, 1:2], win_sb[0:1, 0:1])
            nc.scalar.copy(dummy[0:1, 2:3], fA[0:1, 0:1, 0:1])
            nc.scalar.copy(dummy[0:1, 3:4], fB[0:1, 0:1, 0:1])

            # ---- wavefront conv + interleaved im2col + einsum --------
            # l: 0:x->fA  1:fA->fB  2:fB->fA  3:fA->fB
            fins = [xv, fA, fB, fA]
            fouts = [fA, fB, fA, fB]

            def compact_rows(r0, r1):
                for n in range(4):
                    nc.sync.dma_start(
                        out=f4c[16 * n:16 * n + 16, r0:r1, :],
                        in_=fB[32 * n:32 * n + 16, r0:r1, :])

            def transpose_rows(rr):
                # r in fT2 coords (f4 row = r+3)
                for r in rr:
                    for hf in range(2):
                        tpf = xps.tile([128, 128], f32, tag="tps",
                                       name="tpf")
                        tp = tpf.bitcast(bf)
                        for kw in range(3):
                            nc.tensor.transpose(
                                tp[:, 64 * kw:64 * (kw + 1)],
                                f4c[:, r + 3, 128 * hf + kw:
                                    128 * hf + kw + 128],
                                idbf[:, :])
                        nc.vector.tensor_copy(
                            fT2h[hf][:, 3 * r * 64:3 * (r + 1) * 64],
                            tp[:, 0:192])

            def conv_chunk(l, ch):
                fin, fout = fins[l], fouts[l]
                r0 = 1 + 2 * ch
                ps = cps.tile([128, 2, 256], f32, tag="convps")
                for tap in range(9):
                    kh, kw = tap // 3, tap % 3
                    if l == 0:
                        lhsT = cw[0:12, 0, tap, :]
                        rhs = fin[0:12, r0 + kh - 1:r0 + kh + 1,
                                  kw:kw + 256]
                    else:
                        lhsT = cw[:, l, tap, :]
                        rhs = fin[:, r0 + kh - 1:r0 + kh + 1,
                                  kw:kw + 256]
                    nc.tensor.matmul(
                        ps[:, :, :], lhsT, rhs,
                        start=(tap == 0), stop=(tap == 8))
                nc.scalar.activation(
                    fout[:, r0:r0 + 2, 1:257], ps[:, :, :],
                    mybir.ActivationFunctionType.Relu,
                    bias=cb[:, l:l + 1], scale=1.0)

            def einsum_pc(q, pc):
                pos_t = pos_p.tile([3, 1024], bf, tag="pos")
                nc.scalar.dma_start(
                    pos_t[:, :],
                    post[q, :, pc * 1024:(pc + 1) * 1024])
                hT = ht_p.tile([128, 2, 1024], bf, tag="ht")
                for jh in range(2):
                    for hf2 in range(2):
                        hp = hps.tile([128, 512], f32, tag="hps")
                        nc.tensor.matmul(
                            hp[:, :],
                            w1v[:, jh * 128:(jh + 1) * 128],
                            pos_t[:, hf2 * 512:(hf2 + 1) * 512],
                            start=True, stop=True)
                        nc.scalar.activation(
                            hT[:, jh, hf2 * 512:(hf2 + 1) * 512],
                            hp[:, :],
                            mybir.ActivationFunctionType.Relu,
                            bias=b1c[:, jh:jh + 1], scale=1.0)
                for tl in range(8):
                    t = pc * 8 + tl
                    r0, hf = t // 2, t % 2
                    lwp = lps.tile([128, 3, 9, 16], f32, tag="lwp")
                    for jh in range(2):
                        nc.tensor.matmul(
                            lwp[:, :, :, :],
                            hT[:, jh, tl * 128:(tl + 1) * 128],
                            w2pv[:, jh, :],
                            start=(jh == 0), stop=False)
                    nc.tensor.matmul(
                        lwp[:, :, :, :], ones_sb[:, :], b2pv,
                        start=False, stop=True)
                    lws = lws_p.tile([128, 3, 9, 16], bf, tag="lws")
                    nc.scalar.activation(
                        lws[:, :, :, :], lwp[:, :, :, :],
                        mybir.ActivationFunctionType.Copy)
                    # DVE: STT+accum for 9 pairs (c<2, + c2n0)
                    for c in range(3):
                        for n in range(4):
                            if c == 2 and n > 0:
                                continue
                            scr = scr_p.tile([128, 9, 16], bf,
                                             tag="scr")
                            nc.vector.scalar_tensor_tensor(
                                out=scr[:, :, :],
                                in0=fT2v[hf][:, 3 * r0:3 * r0 + 9,
                                             16 * n:16 * n + 16],
                                scalar=1.0,
                                in1=lws[:, c, :, :],
                                op0=mul, op1=mul,
                                accum_out=outq[q][
                                    :, (n * 3 + c) * 64 + t:
                                    (n * 3 + c) * 64 + t + 1])
                    # DVE: one product for (c=2, n=1..3) pairs
                    scr2 = scr_p.tile([128, 3, 9, 16], bf,
                                      tag="scr2")
                    in0 = fT2v[hf][:, 3 * r0:3 * r0 + 9, :] \
                        .rearrange("p t (n i) -> p t n i", n=4) \
                        .transpose([0, 2, 1, 3])
                    nc.vector.tensor_tensor(
                        out=scr2[:, :, :, :],
                        in0=in0[:, 1:4, :, :],
                        in1=lws[:, 2, :, :].unsqueeze(1)
                        .broadcast_to([128, 3, 9, 16]), op=mul)
                    # ACT: accum-copies for those 3 pairs
                    for n in range(1, 4):
                        aj = aj_p.tile([128, 9, 16], bf, tag="aj")
                        nc.scalar.activation(
                            aj[:, :, :],
                            scr2[:, n - 1, :, :],
                            mybir.ActivationFunctionType.Copy,
                            accum_out=outq[q][
                                :, (n * 3 + 2) * 64 + t:
                                (n * 3 + 2) * 64 + t + 1])

            def writeback(q):
                si, sj = q // 2, q % 2
                for j in range(6):
                    tq = xps.tile([128, 128], f32, tag="tps")
                    nc.tensor.transpose(
                        tq[:, :], outq[q][:, 128 * j:128 * (j + 1)],
                        idf32[:, :])
                    nc.scalar.activation(
                        staged[si].rearrange(
                            "p j (w s) -> p j w s", s=2)[:, j, :, sj],
                        tq[:, :],
                        mybir.ActivationFunctionType.Identity,
                        bias=shiftv[:, j:j + 1], scale=1.0)
                if sj == 1:
                    dstv = outd.rearrange(
                        "n c (r s) (h w) -> (n c) s r h w", s=2, h=2)
                    for j in range(6):
                        nc.sync.dma_start(
                            out=dstv[2 * j:2 * j + 2, si, :, :, :],
                            in_=staged[si][:, j, :])

            fT2v = [tt.rearrange("p (t x) -> p t x", x=64)
                    for tt in fT2h]
            # post-step actions: (after wavefront step s) -> emit list
            def pcgrp(pc):
                return lambda: [einsum_pc(q, pc) for q in range(4)]
            post_step = {
                9: [lambda: compact_rows(3, 9)],
                10: [lambda: transpose_rows(range(0, 6)), pcgrp(0)],
                11: [lambda: compact_rows(9, 13)],
                12: [lambda: transpose_rows(range(6, 10)), pcgrp(1)],
                13: [lambda: compact_rows(13, 17)],
                14: [lambda: transpose_rows(range(10, 14)), pcgrp(2)],
                15: [lambda: compact_rows(17, 21)],
                16: [lambda: transpose_rows(range(14, 18)), pcgrp(3)],
                17: [lambda: compact_rows(21, 25)],
                18: [lambda: transpose_rows(range(18, 22)), pcgrp(4)],
                19: [lambda: compact_rows(25, 29)],
                20: [lambda: transpose_rows(range(22, 26)), pcgrp(5)],
                21: [lambda: compact_rows(29, 33)],
                22: [lambda: transpose_rows(range(26, 30)), pcgrp(6)],
                23: [lambda: compact_rows(33, 37)],
            }
            with tc.tile_pool(name="cps", bufs=2, space="PSUM") as cps, \
                 tc.tile_pool(name="xps", bufs=2, space="PSUM") as xps, \
                 tc.tile_pool(name="hps", bufs=2, space="PSUM") as hps, \
                 tc.tile_pool(name="lps", bufs=2, space="PSUM") as lps:
                for step in range(25):
                    for l in range(4):
                        ch = step - 2 * l
                        if 0 <= ch < 19:
                            conv_chunk(l, ch)
                    for fn in post_step.get(step, []):
                        fn()
                transpose_rows(range(30, 34))
                for q in range(4):
                    einsum_pc(q, 7)
                for q in range(4):
                    writeback(q)
    _legalize_waits(nc)
    return nc


def _get_nc():
    global _NC
    if _NC is None:
        _NC = _build_program()
    return _NC


def _prep_inputs(x, pos_mat, c0w, c0b, c1w, c1b, c2w, c2b, c3w, c3b,
                 w1, b1, w2, b2):
    """Host-side packing of per-core input dicts."""
    x = np.asarray(x, np.float32)
    pos = np.asarray(pos_mat, np.float32).reshape(-1, 3)

    # block-diagonal conv weights cwB[p, l, tap, 32n+co]
    cwB = np.zeros((128, 4, 9, 128), np.float32)
    cbp = np.zeros((128, 4), np.float32)
    for l, (wl, bl) in enumerate(((c0w, c0b), (c1w, c1b),
                                  (c2w, c2b), (c3w, c3b))):
        wl = np.asarray(wl, np.float32)          # (co, ci, 3, 3)
        K = wl.shape[1]
        t = wl.transpose(1, 2, 3, 0).reshape(K, 9, 16)   # (ci, tap, co)
        for n in range(4):
            if l == 0:
                cwB[3 * n:3 * n + K, l, :, 32 * n:32 * n + 16] = t
            else:
                cwB[32 * n:32 * n + K, l, :, 32 * n:32 * n + 16] = t
            cbp[32 * n:32 * n + 16, l] = np.asarray(bl, np.float32)

    w1 = np.asarray(w1, np.float32)              # (3, 256)
    b1p = np.asarray(b1, np.float32).reshape(2, 128).T.copy()  # [j, jh]

    # w2 columns: orig (s=ci*9+tap, c) -> permuted (c, tap, ci)
    w2 = np.asarray(w2, np.float32).reshape(256, 16, 9, 3)     # j,ci,tap,c
    w2pm = w2.transpose(0, 3, 2, 1).reshape(256, 432)          # j,(c,t,ci)
    w2pk = w2pm.reshape(2, 128, 432)                           # [jh,j,432]
    w2pk = np.ascontiguousarray(w2pk.transpose(1, 0, 2))       # [j,jh,432]
    b2 = np.asarray(b2, np.float32).reshape(16, 9, 3)
    b2pk = b2.transpose(2, 1, 0).reshape(432)

    # win pack: [cwB | w2p | b2p]
    winpk = np.zeros((128, WW), np.float32)
    winpk[:, 0:4608] = cwB.reshape(128, 4608)
    winpk[:, 4608:5472] = w2pk.reshape(128, 864)
    winpk[:, 5472:5904] = b2pk[None, :]

    # fin32: [cb | b1c | shift(j)]
    f32pk = np.zeros((128, 12), np.float32)
    f32pk[:, 0:4] = cbp
    f32pk[:, 4:6] = b1p
    for j in range(6):
        for p in range(128):
            nci = 2 * j + (1 if p >= 64 else 0)
            f32pk[p, 6 + j] = RGB_RANGE * RGB_MEAN[nci % 3]

    # pos rows ordered (h, si, w, sj); per-core chunk -> (q, 3, NPIX)
    posr = pos.reshape(Himg, 2, Wimg, 2, 3)

    in_maps = []
    for core in range(NCORES):
        h0 = core * ROWS
        xh = np.zeros((12, NR, WP), np.float32)
        lo, hi = h0 - HALO, h0 + ROWS + HALO
        slo, shi = max(lo, 0), min(hi, Himg)
        for n in range(4):
            xh[3 * n:3 * n + 3, slo - lo:shi - lo, 1:257] = \
                x[n, :, slo:shi, :]
        xpack = np.zeros((12, XW), np.float32)
        xpack[:, :NR * WP] = xh.reshape(12, -1)
        xpack[0:3, NR * WP:] = w1
        pc = posr[h0:h0 + ROWS].transpose(1, 3, 4, 0, 2)  # si,sj,3,h,w
        pc = pc.reshape(4, 3, NPIX)
        in_maps.append({
            "xin": xpack.astype(BF16),
            "win": winpk.astype(BF16),
            "fin32": f32pk,
            "post": np.ascontiguousarray(pc).astype(BF16),
        })
    return in_maps


LAST_RESULTS = None
TRACE = False


def kernel(**inputs):
    global LAST_RESULTS
    nc = _get_nc()
    in_maps = _prep_inputs(**inputs)
    res = run_bass_kernel_spmd(nc, in_maps, core_ids=list(range(NCORES)),
                               trace=TRACE)
    LAST_RESULTS = res
    out = np.concatenate([res.results[i]["out"] for i in range(NCORES)],
                         axis=2)
    return out.astype(np.float32)


# revision 32
# speedup vs baseline: 1.3809x; 1.0538x over previous
"""MetaQuickSR Trainium2 kernel (8-core SPMD, row-sharded), v2.

Sharding: H=256 output-feature rows split 32/core (+4-row conv halo).
Each core: 4-layer CNN (block-diagonal image batching on PE) -> PE-based
im2col row transposes -> bf16 Pos2Weight MLP -> per-pixel locally-
connected einsum split across DVE+Pool -> transpose/interleave writeback
with contiguous output DMAs.  No cross-core communication.
"""

import numpy as np
import ml_dtypes

import concourse.bass as bass
import concourse.mybir as mybir
from concourse.tile import TileContext
from concourse.bass_utils import run_bass_kernel_spmd
from concourse.masks import make_identity

BF16 = ml_dtypes.bfloat16

NCORES = 8
N, CI, Himg, Wimg, S = 4, 16, 256, 256, 2
ROWS = Himg // NCORES          # 32 output-feature rows per core
HALO = 4
NR = ROWS + 2 * HALO           # 40 buffered rows
WP = Wimg + 2                  # 258 zero-padded width
NPIX = ROWS * Wimg             # 8192 einsum pixels per core
NT = NPIX // 128               # 64 pixel tiles per q plane
RGB_MEAN = (0.4488, 0.4371, 0.404)
RGB_RANGE = 255.0

XW = NR * WP + 256             # dense x (12 parts) + w1 rows 0-2
WW = 4 * 9 * 128 + 2 * 432 + 432   # cwB + w2p + b2p(row0)

# einsum reduction split (after a Pool ci-fold halves the volume):
# DVE seg-reduces c < DVE_C in one op plus n < EXTRA_N of c=DVE_C in a
# second; ACT accum-copies the remaining (c, n) pairs.
DVE_C = 2
EXTRA_N = 2

_NC = None


def _legalize_waits(nc, lim=1):
    """This walrus build accepts only one sync-wait per instruction; move
    surplus waits onto same-engine NoOps inserted just before."""
    cnt = 0
    for f in nc.m.functions:
        for bb in f.blocks:
            new = []
            for inst in bb.instructions:
                si = inst.sync_info
                if si is not None and si.on_wait is not None \
                        and len(si.on_wait) > lim:
                    waits = list(si.on_wait)
                    excess, keep = waits[:-lim], waits[-lim:]
                    for w in excess:
                        cnt += 1
                        nop = mybir.InstNoOp(
                            name=f"I-lw{cnt}", opcode="NoOp",
                            engine=inst.engine, debug=inst.debug,
                            ins=[], outs=[],
                            sync_info=mybir.SyncInfo(on_wait=[w],
                                                     on_update=[]))
                        new.append(nop)
                        nc.inst_map[nop.name] = nop
                    inst.sync_info = mybir.SyncInfo(
                        on_wait=keep, on_update=list(si.on_update or []))
                new.append(inst)
            bb.instructions = new
    return cnt


def _build_program():
    nc = bass.Bass(trn_type="TRN2")
    f32 = mybir.dt.float32
    bf = mybir.dt.bfloat16

    xin = nc.dram_tensor("xin", [12, XW], bf, kind="ExternalInput")
    win = nc.dram_tensor("win", [128, WW], bf, kind="ExternalInput")
    fin32 = nc.dram_tensor("fin32", [128, 12], f32, kind="ExternalInput")
    post = nc.dram_tensor("post", [4, 3, NPIX], bf, kind="ExternalInput")
    outd = nc.dram_tensor("out", [4, 3, 2 * ROWS, 2 * Wimg], f32,
                          kind="ExternalOutput")

    mul = mybir.AluOpType.mult

    with TileContext(nc) as tc:
        with (
            tc.tile_pool(name="singles", bufs=1) as singles,
            tc.tile_pool(name="pos_p", bufs=4) as pos_p,
            tc.tile_pool(name="ht_p", bufs=3) as ht_p,
            tc.tile_pool(name="lws_p", bufs=6) as lws_p,
            tc.tile_pool(name="scr_p", bufs=6) as scr_p,
            tc.tile_pool(name="aj_p", bufs=6) as aj_p,
        ):
            # ---- resident tiles --------------------------------------
            xw_sb = singles.tile([12, XW], bf)
            win_sb = singles.tile([128, WW], bf)
            f32_sb = singles.tile([128, 12], f32)
            fA = singles.tile([128, NR, WP], bf)
            fB = singles.tile([128, NR, WP], bf)
            f4c = singles.tile([64, NR, WP], bf)
            # fT2h[hf][pix, (n 4, row 34, kw 3, ci 16)] -- per-image
            # contiguous 144-el tap runs for the einsum operands
            fT2h = [singles.tile([128, 34 * 3 * 64], bf, name=f"fT2h{h}")
                    for h in range(2)]
            outq = [singles.tile([128, 768], f32, name=f"outq{q}")
                    for q in range(4)]
            staged = [singles.tile([128, 6, 256], f32, name=f"stg{s}")
                      for s in range(2)]
            ones_sb = singles.tile([1, 128], bf)
            idbf = singles.tile([64, 64], bf)
            idf32 = singles.tile([128, 128], f32)
            dummy = singles.tile([1, 16], bf)

            nc.sync.dma_start(xw_sb[:, :], xin[:, :])
            nc.scalar.dma_start(win_sb[:, :], win[:, :])
            nc.scalar.dma_start(f32_sb[:, :], fin32[:, :])
            nc.gpsimd.memset(fA[:, :, :], 0.0)
            nc.gpsimd.memset(fB[:, :, :], 0.0)
            nc.gpsimd.memset(ones_sb[:, :], 1.0)
            nc.gpsimd.memset(staged[0][:, :, :], 0.0)
            nc.gpsimd.memset(staged[1][:, :, :], 0.0)
            make_identity(nc, idbf)
            make_identity(nc, idf32)

            xv = xw_sb[:, 0:NR * WP].rearrange("p (r w) -> p r w", w=WP)
            w1v = xw_sb[0:3, NR * WP:NR * WP + 256]
            cw = win_sb[:, 0:4608].rearrange("p (l t o) -> p l t o",
                                             t=9, o=128)
            w2pv = win_sb[:, 4608:4608 + 864].rearrange(
                "p (j c) -> p j c", c=432)
            b2pv = win_sb[0:1, 5472:5904]
            cb = f32_sb[:, 0:4]
            b1c = f32_sb[:, 4:6]
            shiftv = f32_sb[:, 6:12]

            # warm ACT's vector clock (1 wait per op) so conv relu-copies
            # only ever wait on PE.
            nc.scalar.copy(dummy[0:1, 0:1], xw_sb[0:1, 0:1])
            nc.scalar.copy(dummy[0:1, 1:2], win_sb[0:1, 0:1])
            nc.scalar.copy(dummy[0:1, 2:3], fA[0:1, 0:1, 0:1])
            nc.scalar.copy(dummy[0:1, 3:4], fB[0:1, 0:1, 0:1])

            # ---- wavefront conv + interleaved im2col + einsum --------
            # l: 0:x->fA  1:fA->fB  2:fB->fA  3:fA->fB
            fins = [xv, fA, fB, fA]
            fouts = [fA, fB, fA, fB]

            def compact_rows(r0, r1):
                for n in range(4):
                    nc.sync.dma_start(
                        out=f4c[16 * n:16 * n + 16, r0:r1, :],
                        in_=fB[32 * n:32 * n + 16, r0:r1, :])

            def transpose_rows(rr):
                # r in fT2 coords (f4 row = r+3)
                for r in rr:
                    for hf in range(2):
                        tpf = xps.tile([128, 128], f32, tag="tps",
                                       name="tpf")
                        tp = tpf.bitcast(bf)
                        for kw in range(3):
                            nc.tensor.transpose(
                                tp[:, 64 * kw:64 * (kw + 1)],
                                f4c[:, r + 3, 128 * hf + kw:
                                    128 * hf + kw + 128],
                                idbf[:, :])
                        nc.vector.tensor_copy(
                            fT2n[hf][:, :, r, :, :],
                            tp[:, 0:192].rearrange(
                                "p (k n i) -> p n k i", n=4, i=16))

            def conv_chunk(l, ch):
                fin, fout = fins[l], fouts[l]
                r0 = 1 + 2 * ch
                ps = cps.tile([128, 2, 256], f32, tag="convps")
                for tap in range(9):
                    kh, kw = tap // 3, tap % 3
                    if l == 0:
                        lhsT = cw[0:12, 0, tap, :]
                        rhs = fin[0:12, r0 + kh - 1:r0 + kh + 1,
                                  kw:kw + 256]
                    else:
                        lhsT = cw[:, l, tap, :]
                        rhs = fin[:, r0 + kh - 1:r0 + kh + 1,
                                  kw:kw + 256]
                    nc.tensor.matmul(
                        ps[:, :, :], lhsT, rhs,
                        start=(tap == 0), stop=(tap == 8))
                nc.scalar.activation(
                    fout[:, r0:r0 + 2, 1:257], ps[:, :, :],
                    mybir.ActivationFunctionType.Relu,
                    bias=cb[:, l:l + 1], scale=1.0)

            def einsum_pc(q, pc):
                pos_t = pos_p.tile([3, 1024], bf, tag="pos")
                nc.scalar.dma_start(
                    pos_t[:, :],
                    post[q, :, pc * 1024:(pc + 1) * 1024])
                hT = ht_p.tile([128, 2, 1024], bf, tag="ht")
                for jh in range(2):
                    for hf2 in range(2):
                        hp = hps.tile([128, 512], f32, tag="hps")
                        nc.tensor.matmul(
                            hp[:, :],
                            w1v[:, jh * 128:(jh + 1) * 128],
                            pos_t[:, hf2 * 512:(hf2 + 1) * 512],
                            start=True, stop=True)
                        nc.scalar.activation(
                            hT[:, jh, hf2 * 512:(hf2 + 1) * 512],
                            hp[:, :],
                            mybir.ActivationFunctionType.Relu,
                            bias=b1c[:, jh:jh + 1], scale=1.0)
                for tl in range(8):
                    t = pc * 8 + tl
                    r0, hf = t // 2, t % 2
                    lwp = lps.tile([128, 3, 9, 16], f32, tag="lwp")
                    for jh in range(2):
                        nc.tensor.matmul(
                            lwp[:, :, :, :],
                            hT[:, jh, tl * 128:(tl + 1) * 128],
                            w2pv[:, jh, :],
                            start=(jh == 0), stop=False)
                    nc.tensor.matmul(
                        lwp[:, :, :, :], ones_sb[:, :], b2pv,
                        start=False, stop=True)
                    lws = lws_p.tile([128, 3, 9, 16], bf, tag="lws")
                    if tl % 4 == 3:
                        nc.vector.tensor_copy(lws[:, :, :, :],
                                              lwp[:, :, :, :])
                    else:
                        nc.scalar.activation(
                            lws[:, :, :, :], lwp[:, :, :, :],
                            mybir.ActivationFunctionType.Copy)
                    # DVE: STT+accum for 9 pairs (c<2, + c2n0)
                    for c in range(3):
                        for n in range(4):
                            if c == 2 and n > 0:
                                continue
                            scr = scr_p.tile([128, 9, 16], bf,
                                             tag="scr")
                            nc.vector.scalar_tensor_tensor(
                                out=scr[:, :, :],
                                in0=fT2m[hf][:, n,
                                             3 * r0:3 * r0 + 9, :],
                                scalar=1.0,
                                in1=lws[:, c, :, :],
                                op0=mul, op1=mul,
                                accum_out=outq[q][
                                    :, (n * 3 + c) * 64 + t:
                                    (n * 3 + c) * 64 + t + 1])
                    # DVE: one product for (c=2, n=1..3) pairs
                    scr2 = scr_p.tile([128, 3, 9, 16], bf,
                                      tag="scr2")
                    in0 = fT2m[hf][:, :, 3 * r0:3 * r0 + 9, :]
                    nc.vector.tensor_tensor(
                        out=scr2[:, :, :, :],
                        in0=in0[:, 1:4, :, :],
                        in1=lws[:, 2, :, :].unsqueeze(1)
                        .broadcast_to([128, 3, 9, 16]), op=mul)
                    # ACT: accum-copies for those 3 pairs
                    for n in range(1, 4):
                        aj = aj_p.tile([128, 9, 16], bf, tag="aj")
                        nc.scalar.activation(
                            aj[:, :, :],
                            scr2[:, n - 1, :, :],
                            mybir.ActivationFunctionType.Copy,
                            accum_out=outq[q][
                                :, (n * 3 + 2) * 64 + t:
                                (n * 3 + 2) * 64 + t + 1])

            def writeback(q):
                si, sj = q // 2, q % 2
                for j in range(6):
                    tq = xps.tile([128, 128], f32, tag="tps")
                    nc.tensor.transpose(
                        tq[:, :], outq[q][:, 128 * j:128 * (j + 1)],
                        idf32[:, :])
                    nc.scalar.activation(
                        staged[si].rearrange(
                            "p j (w s) -> p j w s", s=2)[:, j, :, sj],
                        tq[:, :],
                        mybir.ActivationFunctionType.Identity,
                        bias=shiftv[:, j:j + 1], scale=1.0)
                if sj == 1:
                    dstv = outd.rearrange(
                        "n c (r s) (h w) -> (n c) s r h w", s=2, h=2)
                    for j in range(6):
                        nc.sync.dma_start(
                            out=dstv[2 * j:2 * j + 2, si, :, :, :],
                            in_=staged[si][:, j, :])

            fT2n = [tt.rearrange("p (n r k i) -> p n r k i", n=4,
                                 k=3, i=16) for tt in fT2h]
            fT2m = [tt.rearrange("p (n rk i) -> p n rk i", n=4, i=16)
                    for tt in fT2h]
            # post-step actions: (after wavefront step s) -> emit list
            def pcgrp(pc):
                return lambda: [einsum_pc(q, pc) for q in range(4)]
            post_step = {
                9: [lambda: compact_rows(3, 9)],
                10: [lambda: transpose_rows(range(0, 6)), pcgrp(0)],
                11: [lambda: compact_rows(9, 13)],
                12: [lambda: transpose_rows(range(6, 10)), pcgrp(1)],
                13: [lambda: compact_rows(13, 17)],
                14: [lambda: transpose_rows(range(10, 14)), pcgrp(2)],
                15: [lambda: compact_rows(17, 21)],
                16: [lambda: transpose_rows(range(14, 18)), pcgrp(3)],
                17: [lambda: compact_rows(21, 25)],
                18: [lambda: transpose_rows(range(18, 22)), pcgrp(4)],
                19: [lambda: compact_rows(25, 29)],
                20: [lambda: transpose_rows(range(22, 26)), pcgrp(5)],
                21: [lambda: compact_rows(29, 33)],
                22: [lambda: transpose_rows(range(26, 30)), pcgrp(6)],
                23: [lambda: compact_rows(33, 37)],
            }
            with tc.tile_pool(name="cps", bufs=2, space="PSUM") as cps, \
                 tc.tile_pool(name="xps", bufs=2, space="PSUM") as xps, \
                 tc.tile_pool(name="hps", bufs=2, space="PSUM") as hps, \
                 tc.tile_pool(name="lps", bufs=2, space="PSUM") as lps:
                for step in range(25):
                    for l in range(4):
                        ch = step - 2 * l
                        if 0 <= ch < 19:
                            conv_chunk(l, ch)
                    for fn in post_step.get(step, []):
                        fn()
                transpose_rows(range(30, 34))
                for q in range(4):
                    einsum_pc(q, 7)
                for q in range(4):
                    writeback(q)
    _legalize_waits(nc)
    return nc


def _get_nc():
    global _NC
    if _NC is None:
        _NC = _build_program()
    return _NC


def _prep_inputs(x, pos_mat, c0w, c0b, c1w, c1b, c2w, c2b, c3w, c3b,
                 w1, b1, w2, b2):
    """Host-side packing of per-core input dicts."""
    x = np.asarray(x, np.float32)
    pos = np.asarray(pos_mat, np.float32).reshape(-1, 3)

    # block-diagonal conv weights cwB[p, l, tap, 32n+co]
    cwB = np.zeros((128, 4, 9, 128), np.float32)
    cbp = np.zeros((128, 4), np.float32)
    for l, (wl, bl) in enumerate(((c0w, c0b), (c1w, c1b),
                                  (c2w, c2b), (c3w, c3b))):
        wl = np.asarray(wl, np.float32)          # (co, ci, 3, 3)
        K = wl.shape[1]
        t = wl.transpose(1, 2, 3, 0).reshape(K, 9, 16)   # (ci, tap, co)
        for n in range(4):
            if l == 0:
                cwB[3 * n:3 * n + K, l, :, 32 * n:32 * n + 16] = t
            else:
                cwB[32 * n:32 * n + K, l, :, 32 * n:32 * n + 16] = t
            cbp[32 * n:32 * n + 16, l] = np.asarray(bl, np.float32)

    w1 = np.asarray(w1, np.float32)              # (3, 256)
    b1p = np.asarray(b1, np.float32).reshape(2, 128).T.copy()  # [j, jh]

    # w2 columns: orig (s=ci*9+tap, c) -> permuted (c, tap, ci)
    w2 = np.asarray(w2, np.float32).reshape(256, 16, 9, 3)     # j,ci,tap,c
    w2pm = w2.transpose(0, 3, 2, 1).reshape(256, 432)          # j,(c,t,ci)
    w2pk = w2pm.reshape(2, 128, 432)                           # [jh,j,432]
    w2pk = np.ascontiguousarray(w2pk.transpose(1, 0, 2))       # [j,jh,432]
    b2 = np.asarray(b2, np.float32).reshape(16, 9, 3)
    b2pk = b2.transpose(2, 1, 0).reshape(432)

    # win pack: [cwB | w2p | b2p]
    winpk = np.zeros((128, WW), np.float32)
    winpk[:, 0:4608] = cwB.reshape(128, 4608)
    winpk[:, 4608:5472] = w2pk.reshape(128, 864)
    winpk[:, 5472:5904] = b2pk[None, :]

    # fin32: [cb | b1c | shift(j)]
    f32pk = np.zeros((128, 12), np.float32)
    f32pk[:, 0:4] = cbp
    f32pk[:, 4:6] = b1p
    for j in range(6):
        for p in range(128):
            nci = 2 * j + (1 if p >= 64 else 0)
            f32pk[p, 6 + j] = RGB_RANGE * RGB_MEAN[nci % 3]

    # pos rows ordered (h, si, w, sj); per-core chunk -> (q, 3, NPIX)
    posr = pos.reshape(Himg, 2, Wimg, 2, 3)

    in_maps = []
    for core in range(NCORES):
        h0 = core * ROWS
        xh = np.zeros((12, NR, WP), np.float32)
        lo, hi = h0 - HALO, h0 + ROWS + HALO
        slo, shi = max(lo, 0), min(hi, Himg)
        for n in range(4):
            xh[3 * n:3 * n + 3, slo - lo:shi - lo, 1:257] = \
                x[n, :, slo:shi, :]
        xpack = np.zeros((12, XW), np.float32)
        xpack[:, :NR * WP] = xh.reshape(12, -1)
        xpack[0:3, NR * WP:] = w1
        pc = posr[h0:h0 + ROWS].transpose(1, 3, 4, 0, 2)  # si,sj,3,h,w
        pc = pc.reshape(4, 3, NPIX)
        in_maps.append({
            "xin": xpack.astype(BF16),
            "win": winpk.astype(BF16),
            "fin32": f32pk,
            "post": np.ascontiguousarray(pc).astype(BF16),
        })
    return in_maps


LAST_RESULTS = None
TRACE = False


def kernel(**inputs):
    global LAST_RESULTS
    nc = _get_nc()
    in_maps = _prep_inputs(**inputs)
    res = run_bass_kernel_spmd(nc, in_maps, core_ids=list(range(NCORES)),
                               trace=TRACE)
    LAST_RESULTS = res
    out = np.concatenate([res.results[i]["out"] for i in range(NCORES)],
                         axis=2)
    return out.astype(np.float32)
